# revision 27
# baseline (speedup 1.0000x reference)
"""Trainium2 Bass kernel for nn_Allocator2 (dense_cnn), 8 NeuronCores.

Pure data parallelism: batch 64 -> 8 samples per core, weights replicated.

v2 redesign around PE array packing + a skewed windowed pipeline:

  head : 1x1 convs packed across 8 samples with block-diagonal weights,
         PSUM borrowed from the stage tiles.
  dil  : output in hh-major layout, two chunks with a duplicated row band:
           A = (hh 0..3) x 25 branches = 100 rows (r = 25*hh + i)
           B = (hh 3..6) x 25 branches = 100 rows
         row-packed: A at array rows 0..51, B at rows 64..115 (S and the
         B weights are duplicated at partitions 64+), so the two matmuls
         run concurrently in disjoint row groups.
  F1   : two chains, h in {0,1,2} on chunk A and h in {3,4,5} on chunk B,
         each K=100, M=48 (m = 16*h_loc + o).  The chains share the same
         lhsT (translation invariance in h) and run concurrently in col
         groups (out partitions 0..47 / 64..111), interleaved
         accumulation chains in one PSUM bank pair.
  F2   : K=112 over the gapped a1 layout (zero weights in rows 48..63),
         M=40 (m = 5*o + h), 2-sample col packing (out partitions 0..39
         and 64..103), one shared a2 pair tile + single ACT.
  F3   : baked K=120 (3 phases) x 2 taps, 2-sample col packing, single
         vector threshold per tile (round(sigmoid) == z > -bF3).

  Stage buffers are ~1KB ring slots stepped by STEP=1008 with cascaded
  extra columns (dil +15, F1 +10, F2 +5, bake +3) so each consumer tile j
  reads only producer slot j.  Global skewed pipeline over 36 (pair,
  supertile) ticks: tick g runs dil(g), F1(g-1), F2+bake(g-2), F3(g-3),
  so the PE never drains between stages, samples, or pairs.
"""

import numpy as np
import ml_dtypes

BF16 = ml_dtypes.bfloat16

B = 64            # global batch
NCORES = 8
BS = B // NCORES  # 8 samples per core
ND = 25
L = 8192          # concat length (4096 + 4096)
LX = 4096
LC = L - ND       # 8167 dilated output length
T1 = LC - 5       # 8162 F1 output length
T2 = T1 - 5       # 8157
T3 = T2 - 5       # 8152
NT = 512          # matmul free-dim tile
STEP = 1008       # supertile step
WIN = 1024        # ring slot width (2 PSUM banks of fp32)
NSUP = -(-LC // STEP)   # 9 supertiles per sample
NPAIR = BS // 2


def _bd(blocks):
    """block-diagonal stack of 2D arrays"""
    rs = sum(b.shape[0] for b in blocks)
    cs = sum(b.shape[1] for b in blocks)
    out = np.zeros((rs, cs), np.float32)
    r = c = 0
    for b in blocks:
        out[r:r + b.shape[0], c:c + b.shape[1]] = b
        r += b.shape[0]
        c += b.shape[1]
    return out


def build_weights(inp):
    """Host-side weight prep. Returns dict of np arrays (bf16 weights,
    fp32 biases) shared by all cores."""
    w = {}
    f32 = np.float32

    # ---- head: block-diagonal over BS samples, lhsT layout [K, M] ----
    def head_lhsT(wmat):  # wmat [Co, Ci] -> lhsT [Ci, Co] per sample
        return _bd([wmat.T.astype(f32)] * BS)

    w['hT1'] = head_lhsT(inp['wT1'])   # [72, 48]
    w['hT2'] = head_lhsT(inp['wT2'])   # [48, 32]
    w['hT3'] = head_lhsT(inp['wT3'])   # [32, 16]
    w['hR1'] = head_lhsT(inp['wR1'])   # [24, 16]
    w['hR2'] = head_lhsT(inp['wR2'])   # [16, 16]
    for nm in ('bT1', 'bT2', 'bT3', 'bR1', 'bR2'):
        w['hb' + nm[1:]] = np.tile(inp[nm].astype(f32), BS)[:, None]

    # ---- dil: original [52, 175], cols remapped to hh-major chunks ----
    dil = np.zeros((52, 175), f32)
    wM = inp['wM'].astype(f32)  # [25, 7, 2, 2]
    for i in range(ND):
        for o in range(7):
            m = i * 7 + o
            for c in range(2):
                dil[c * 26 + 0, m] = wM[i, o, c, 0]          # shift 0 tap
                dil[c * 26 + (i + 1), m] = wM[i, o, c, 1]    # shift i+1 tap
    dilA = np.zeros((52, 100), f32)   # cols 25*o + i, o in 0..3
    dilB = np.zeros((52, 100), f32)   # cols 25*(o-3) + i, o in 3..6
    bMA = np.zeros((100,), f32)
    bMB = np.zeros((100,), f32)
    for i in range(ND):
        for o in range(7):
            if o < 4:
                dilA[:, 25 * o + i] = dil[:, i * 7 + o]
                bMA[25 * o + i] = inp['bM'][i, o]
            if o >= 3:
                dilB[:, 25 * (o - 3) + i] = dil[:, i * 7 + o]
                bMB[25 * (o - 3) + i] = inp['bM'][i, o]
    w['dilA'] = dilA
    w['dilB'] = dilB
    w['bMA'] = bMA[:, None]
    w['bMB'] = bMB[:, None]

    # ---- F1: 6 lhsT [100, 48], shared by both chains ----
    wF1 = inp['wF1'].astype(f32)  # [16, 25, 2, 6]
    f1 = np.zeros((6, 100, 48), f32)
    for dw in range(6):
        for hl in range(3):
            for hhl in range(4):
                dh = hhl - hl
                if 0 <= dh <= 1:
                    for o in range(16):
                        for i in range(ND):
                            f1[dw, 25 * hhl + i, 16 * hl + o] = wF1[o, i, dh, dw]
    w['F1'] = f1
    # bias in a1 layout [112]: h<3 -> 16h+o ; h>=3 -> 64+16(h-3)+o
    bF1 = np.zeros((112,), f32)
    for h in range(6):
        r0 = 16 * h if h < 3 else 64 + 16 * (h - 3)
        for o in range(16):
            bF1[r0 + o] = inp['bF1'][o]
    w['bF1'] = bF1[:, None]

    # ---- F2: 6 lhsT [112, 40] over gapped a1 rows, m = 5*o + h ----
    wF2 = inp['wF2'].astype(f32)  # [8, 16, 2, 6]
    f2 = np.zeros((6, 112, 40), f32)
    for dw in range(6):
        for hh in range(6):
            r0 = 16 * hh if hh < 3 else 64 + 16 * (hh - 3)
            for ci in range(16):
                for o in range(8):
                    for h in range(5):
                        dh = hh - h
                        if 0 <= dh <= 1:
                            f2[dw, r0 + ci, o * 5 + h] = wF2[o, ci, dh, dw]
    w['F2'] = f2
    bF2 = np.zeros((104,), f32)
    for o in range(8):
        for h in range(5):
            bF2[o * 5 + h] = inp['bF2'][o]
            bF2[64 + o * 5 + h] = inp['bF2'][o]
    w['bF2'] = bF2[:, None]

    # ---- F3 direct: lhsT[dw] [40, 4]; row m = ci*5 + hh, col h ----
    wF3 = inp['wF3'].astype(f32)  # [1, 8, 2, 6]
    f3 = np.zeros((6, 40, 4), f32)
    for dw in range(6):
        for ci in range(8):
            for hh in range(5):
                for h in range(4):
                    dh = hh - h
                    if 0 <= dh <= 1:
                        f3[dw, ci * 5 + hh, h] = wF3[0, ci, dh, dw]
    w['F3'] = f3
    thr = np.zeros((68,), f32)
    thr[0:4] = -inp['bF3'][0]
    thr[64:68] = -inp['bF3'][0]
    w['thr'] = thr[:, None]

    # bf16-ify matmul weights
    for k in ('hT1', 'hT2', 'hT3', 'hR1', 'hR2', 'dilA', 'dilB',
              'F1', 'F2', 'F3'):
        w[k] = w[k].astype(BF16)
    return w


def emulate_core(w, x_core, y_core):
    """Numpy emulation of exactly what the Bass kernel computes for one
    core. x_core [72, 4096] bf16, y_core [24, 4096] bf16. Returns
    [BS, 4, T3] f32 in {0,1}."""
    f32 = np.float32

    def mm(lhsT, rhs):  # bf16 operands, f32 accumulate
        return lhsT.astype(f32).T @ rhs.astype(f32)

    relu = lambda a: np.maximum(a, 0)
    sig = lambda a: 1.0 / (1.0 + np.exp(-a))

    a = relu(mm(w['hT1'], x_core) + w['hbT1']).astype(BF16)
    a = relu(mm(w['hT2'], a) + w['hbT2']).astype(BF16)
    t3 = (mm(w['hT3'], a) + w['hbT3']).astype(BF16)          # [16, 4096]
    b_ = relu(mm(w['hR1'], y_core) + w['hbR1']).astype(BF16)
    b_ = relu(mm(w['hR2'], b_) + w['hbR2']).astype(BF16)     # [16, 4096]
    out2 = np.concatenate([t3, b_], axis=1)                  # [16, 8192] bf16

    res = np.zeros((BS, 4, T3), f32)
    for s in range(BS):
        o2 = out2[s * 2:s * 2 + 2]                           # [2, 8192]
        S = np.zeros((52, LC), BF16)
        for c in range(2):
            for sh in range(26):
                S[c * 26 + sh] = o2[c, sh:sh + LC]
        OA = relu(mm(w['dilA'], S) + w['bMA']).astype(BF16)   # [100, LC]
        OB = relu(mm(w['dilB'], S) + w['bMB']).astype(BF16)   # [100, LC]
        z1 = np.zeros((112, T1), f32)
        for dw in range(6):
            z1[0:48] += mm(w['F1'][dw], OA[:, dw:dw + T1])
            z1[64:112] += mm(w['F1'][dw], OB[:, dw:dw + T1])
        a1 = sig(z1 + w['bF1']).astype(BF16)                 # [112, T1]
        a1[48:64] = 0.0                                      # gap rows
        z2 = np.zeros((40, T2), f32)
        for dw in range(6):
            z2 += mm(w['F2'][dw], a1[:, dw:dw + T2])
        a2 = sig(z2 + w['bF2'][0:40]).astype(BF16)           # [40, T2]
        z3 = np.zeros((4, T3), f32)
        for dw in range(6):
            z3 += mm(w['F3'][dw], a2[:, dw:dw + T3])
        res[s] = (z3 > w['thr'][0:4]).astype(f32)            # [4, T3]
    return res


def _shard_inputs(inputs):
    """Build per-core in_maps (host-side prep + shard)."""
    w = build_weights(inputs)
    in_maps = []
    for c in range(NCORES):
        m = dict(w)
        xs = inputs['x'][c * BS:(c + 1) * BS]  # [8, 9, 4096]
        ys = inputs['y'][c * BS:(c + 1) * BS]
        m['x'] = np.ascontiguousarray(xs.reshape(BS * 9, LX)).astype(BF16)
        m['y'] = np.ascontiguousarray(ys.reshape(BS * 3, LX)).astype(BF16)
        in_maps.append(m)
    return in_maps


# ---------------------------------------------------------------------------
# Bass program
# ---------------------------------------------------------------------------

def _split_excess_waits(bir, maxw=1):
    """The walrus build in this container refuses instructions carrying
    more than ~1 semaphore wait ("Too many sync wait commands").  Tile
    attaches multi-waits freely.  Splitting is semantics-preserving: move
    excess waits onto injected NoOps on the same engine immediately
    before the instruction (engines execute their instruction stream in
    order, so wait-all is preserved)."""
    for fn in bir['functions']:
        for bb in fn['blocks']:
            out = []
            for inst in bb['instructions']:
                si = inst.get('sync_info')
                waits = (si or {}).get('on_wait') or []
                if len(waits) > maxw:
                    extra, keep = waits[:-maxw], waits[-maxw:]
                    for i in range(0, len(extra), maxw):
                        out.append({
                            "debug": inst.get("debug", 0),
                            "engine": inst["engine"], "ins": [],
                            "name": f"{inst['name']}-wsplit{i}",
                            "opcode": "NoOp", "outs": [],
                            "sync_info": {"on_update": [],
                                          "on_wait": extra[i:i + maxw]}})
                    si['on_wait'] = keep
                out.append(inst)
            bb['instructions'] = out
    return bir


def _patch_serialization(nc):
    import orjson
    bir = _split_excess_waits(nc.to_json())
    patched = orjson.dumps(bir)
    nc.to_json_bytes = lambda: patched
    return nc


# per-stage output lengths and window extensions
EXT = {'dil': 15, 'f1': 10, 'f2': 5, 'f3': 0}
LST = {'dil': LC, 'f1': T1, 'f2': T2, 'f3': T3}


def _wlen(stage, j):
    """columns stage tile j computes: [STEP*j, STEP*j + wlen)"""
    t0 = STEP * j
    if t0 >= LST[stage]:
        return 0
    return min(t0 + STEP + EXT[stage], LST[stage]) - t0


def build_bass():
    import bass_rust
    import concourse.bass as bass
    import concourse.mybir as mybir
    from concourse.tile import TileContext

    dt = mybir.dt
    AF = mybir.ActivationFunctionType
    ALU = mybir.AluOpType

    nc = bass.Bass()

    p = {}
    p['x'] = nc.declare_dram_parameter('x', [BS * 9, LX], dt.bfloat16, False)
    p['y'] = nc.declare_dram_parameter('y', [BS * 3, LX], dt.bfloat16, False)
    for nm, sh in [('hT1', [BS * 9, BS * 6]), ('hT2', [BS * 6, BS * 4]),
                   ('hT3', [BS * 4, BS * 2]),
                   ('hR1', [BS * 3, BS * 2]), ('hR2', [BS * 2, BS * 2]),
                   ('dilA', [52, 100]), ('dilB', [52, 100]),
                   ('F1', [6, 100, 48]), ('F2', [6, 112, 40]),
                   ('F3', [6, 40, 4])]:
        p[nm] = nc.declare_dram_parameter(nm, sh, dt.bfloat16, False)
    for nm, sh in [('hbT1', [BS * 6, 1]), ('hbT2', [BS * 4, 1]),
                   ('hbT3', [BS * 2, 1]),
                   ('hbR1', [BS * 2, 1]), ('hbR2', [BS * 2, 1]),
                   ('bMA', [100, 1]), ('bMB', [100, 1]),
                   ('bF1', [112, 1]), ('bF2', [104, 1]), ('thr', [68, 1])]:
        p[nm] = nc.declare_dram_parameter(nm, sh, dt.float32, False)
    out_d = nc.declare_dram_parameter('out', [BS * 4, T3], dt.bfloat16, True)
    # DRAM staging for o2t: window reads from HBM avoid the SBUF
    # per-partition read bottleneck (26x overlapping reads of one row)
    o2d = nc.dram_tensor('o2d', [BS * 2, L], dt.bfloat16)

    with TileContext(nc) as tc:
        with tc.tile_pool(name="wpool", bufs=1) as wp, \
             tc.tile_pool(name="head", bufs=1) as hp, \
             tc.tile_pool(name="ring", bufs=1) as rp, \
             tc.tile_pool(name="psum", bufs=1, space="PSUM") as pp:

            # ---------------- weights to SBUF ----------------
            # head weights + inputs on the sync queue (head critical path);
            # stage weights on gpsimd (hidden under the head)
            xt = hp.tile([BS * 9, LX], dt.bfloat16, name="xt")
            yt = hp.tile([BS * 3, LX], dt.bfloat16, name="yt")
            nc.sync.dma_start(out=xt[...], in_=p['x'][...])
            W = {}
            for nm in ('hT1', 'hbT1', 'hR1', 'hbR1', 'hT2', 'hbT2',
                       'hR2', 'hbR2', 'hT3', 'hbT3'):
                t = wp.tile(list(p[nm].shape), p[nm].dtype, name=f"w_{nm}")
                nc.sync.dma_start(out=t[...], in_=p[nm][...])
                W[nm] = t
            for nm in ('dilA', 'bMA', 'bMB', 'bF1', 'bF2', 'thr'):
                t = wp.tile(list(p[nm].shape), p[nm].dtype, name=f"w_{nm}")
                nc.gpsimd.dma_start(out=t[...], in_=p[nm][...])
                W[nm] = t
            # dilB duplicated at partitions 64..115 for row tiling
            w_dilB = wp.tile([116, 100], dt.bfloat16, name="w_dilB")
            nc.gpsimd.dma_start(out=w_dilB[64:116], in_=p['dilB'][...])
            W['dilB'] = w_dilB
            for nm in ('F1', 'F2', 'F3'):
                n_sl, kk, mm_ = p[nm].shape
                W[nm] = []
                for i_sl in range(n_sl):
                    t = wp.tile([kk, mm_], p[nm].dtype, name=f"w_{nm}{i_sl}")
                    nc.gpsimd.dma_start(out=t[...], in_=p[nm][i_sl])
                    W[nm].append(t)
            # F3 weights duplicated at partitions 64..103 for row tiling
            W['F3d'] = []
            for i_sl in range(6):
                t = wp.tile([104, 4], dt.bfloat16, name=f"w_F3d{i_sl}")
                nc.gpsimd.dma_start(out=t[64:104], in_=p['F3'][i_sl])
                W['F3d'].append(t)

            # ---------------- static PSUM tiles (8 banks) ----------------
            P_DA = pp.tile([128, WIN], dt.float32, name="P_DA")    # 2 banks
            P_DB = pp.tile([128, NT], dt.float32, name="P_DB")     # 1 bank
            P_F1 = pp.tile([128, WIN], dt.float32, name="P_F1")    # 2 banks
            P_F2 = pp.tile([128, WIN], dt.float32, name="P_F2")    # 2 banks
            P_F3 = pp.tile([128, NT], dt.float32, name="P_F3")     # 1 bank

            # one-time zero of the F1 gap rows (never matmul-written)
            nc.vector.memset(P_F1[32:64, :], 0.0)

            # ---------------- head ----------------
            nc.sync.dma_start(out=yt[...], in_=p['y'][...])
            o2t = hp.tile([BS * 2, L], dt.bfloat16, name="o2t")
            head_ps = [P_DA, P_F1, P_F2]

            def head_layer(w_nm, b_nm, rows_in, rows_out, src, dst, act,
                           dst_off=0):
                for j in range(LX // WIN):
                    pt = head_ps[j % 3]
                    for h in range(2):
                        sl = slice(j * WIN + h * NT, j * WIN + (h + 1) * NT)
                        nc.tensor.matmul(pt[0:rows_out, h * NT:(h + 1) * NT],
                                         W[w_nm][...], src[:rows_in, sl],
                                         start=True, stop=True)
                    sl2 = slice(dst_off + j * WIN, dst_off + (j + 1) * WIN)
                    ps = pt[0:rows_out, 0:WIN]
                    if act == 'relu':
                        nc.scalar.activation(dst[:rows_out, sl2], ps,
                                             AF.Relu, bias=W[b_nm][...])
                    else:
                        nc.vector.tensor_scalar(dst[:rows_out, sl2], ps,
                                                W[b_nm][...], None, ALU.add)

            a1h = hp.tile([BS * 6, LX], dt.bfloat16, name="a1h")
            a2h = hp.tile([BS * 6, LX], dt.bfloat16, name="a2h")
            b1h = hp.tile([BS * 6, LX], dt.bfloat16, name="b1h")
            head_layer('hT1', 'hbT1', BS * 9, BS * 6, xt, a1h, 'relu')
            head_layer('hT2', 'hbT2', BS * 6, BS * 4, a1h, a2h, 'relu')
            head_layer('hT3', 'hbT3', BS * 4, BS * 2, a2h, o2t, 'add')
            head_layer('hR1', 'hbR1', BS * 3, BS * 2, yt, b1h, 'relu')
            head_layer('hR2', 'hbR2', BS * 2, BS * 2, b1h, o2t, 'relu',
                       dst_off=LX)
            # stage o2t to DRAM for the window builds
            nc.sync.dma_start(out=o2d[...], in_=o2t[...])

            # ---------------- ring buffers ----------------
            S_bufs = [rp.tile([116, LC], dt.bfloat16, name=f"S{i}")
                      for i in range(4)]
            # per-sample rings: slot = 2*(s%2) + (t%2), t = 9*pair + j
            Oa = [rp.tile([100, WIN], dt.bfloat16, name=f"Oa{i}")
                  for i in range(4)]
            Ob = [rp.tile([100, WIN], dt.bfloat16, name=f"Ob{i}")
                  for i in range(4)]
            a1r = [rp.tile([112, WIN], dt.bfloat16, name=f"a1r{i}")
                   for i in range(4)]
            # per-pair rings: slot = t%2
            a2r = [rp.tile([104, WIN], dt.bfloat16, name=f"a2r{i}")
                   for i in range(2)]
            of3 = [rp.tile([68, NT], dt.bfloat16, name=f"of3{i}")
                   for i in range(2)]

            def prefetch_S(s, dup_dram=False):
                """window DMAs (from DRAM staging) building S for sample s:
                primary rows c*26+w split into column halves across queues,
                row-64 duplicate via one SBUF copy (or from DRAM at start)."""
                St = S_bufs[s % 4]
                H2 = LC // 2
                qs = [nc.scalar, nc.sync]
                for c in range(2):
                    for k, (u0, u1) in enumerate(((0, H2), (H2, LC))):
                        win = o2d[s * 2 + c:s * 2 + c + 1, u0:u1].copy()
                        win.ap = bass_rust.VecI64Pair(
                            [[L, 1], [1, 26], [1, u1 - u0]])
                        qs[(c + k) % 2].dma_start(
                            out=St[c * 26:(c + 1) * 26, u0:u1], in_=win)
                        if dup_dram:
                            win2 = o2d[s * 2 + c:s * 2 + c + 1, u0:u1].copy()
                            win2.ap = bass_rust.VecI64Pair(
                                [[L, 1], [1, 26], [1, u1 - u0]])
                            qs[(c + k + 1) % 2].dma_start(
                                out=St[64 + c * 26:64 + (c + 1) * 26, u0:u1],
                                in_=win2)
                if not dup_dram:
                    nc.gpsimd.dma_start(out=St[64:116, :], in_=St[0:52, :])

            def halves(ln):
                out = [(0, min(NT, ln))]
                if ln > NT:
                    out.append((NT, ln))
                return out

            # ---------------- stage emitters ----------------
            def dil_mm(s, j, half):
                ln = _wlen('dil', j)
                hs = halves(ln)
                if half >= len(hs):
                    return
                lo, hi = hs[half]
                St = S_bufs[s % 4]
                t0 = STEP * j
                nc.tensor.matmul(P_DA[0:100, lo:hi], W['dilA'][...],
                                 St[0:52, t0 + lo:t0 + hi],
                                 start=True, stop=True, tile_position=(0, 0),
                                 skip_group_check=True)
                nc.tensor.matmul(P_DB[0:100, 0:hi - lo], W['dilB'][64:116],
                                 St[64:116, t0 + lo:t0 + hi],
                                 start=True, stop=True, tile_position=(64, 0),
                                 skip_group_check=True)

            def dil_post(s, j, slot, half):
                ln = _wlen('dil', j)
                hs = halves(ln)
                if half >= len(hs):
                    return
                lo, hi = hs[half]
                nc.vector.tensor_scalar(Ob[slot][:, lo:hi],
                                        P_DB[0:100, 0:hi - lo],
                                        W['bMB'][...], 0.0, ALU.add, ALU.max)
                if half == len(hs) - 1:
                    nc.scalar.activation(Oa[slot][:, 0:ln], P_DA[0:100, 0:ln],
                                         AF.Relu, bias=W['bMA'][...])

            def f1_mm(s, j, slot, half):
                ln = _wlen('f1', j)
                hs = halves(ln)
                if half >= len(hs):
                    return
                lo, hi = hs[half]
                for dw in range(6):
                    nc.tensor.matmul(P_F1[0:48, lo:hi], W['F1'][dw][...],
                                     Oa[slot][:, lo + dw:hi + dw],
                                     start=(dw == 0), stop=(dw == 5),
                                     skip_group_check=True)
                    nc.tensor.matmul(P_F1[64:112, lo:hi], W['F1'][dw][...],
                                     Ob[slot][:, lo + dw:hi + dw],
                                     start=(dw == 0), stop=(dw == 5),
                                     skip_group_check=True)

            def f1_post(s, j, slot):
                ln = _wlen('f1', j)
                nc.scalar.activation(a1r[slot][:, 0:ln], P_F1[0:112, 0:ln],
                                     AF.Sigmoid, bias=W['bF1'][...])

            def f2_mm(pair, j, t, half):
                ln = _wlen('f2', j)
                hs = halves(ln)
                if half >= len(hs):
                    return
                lo, hi = hs[half]
                sl0 = 2 * 0 + (t % 2)   # sample 2*pair
                sl1 = 2 * 1 + (t % 2)   # sample 2*pair+1
                for dw in range(6):
                    nc.tensor.matmul(P_F2[0:40, lo:hi], W['F2'][dw][...],
                                     a1r[sl0][:, lo + dw:hi + dw],
                                     start=(dw == 0), stop=(dw == 5),
                                     skip_group_check=True)
                    nc.tensor.matmul(P_F2[64:104, lo:hi], W['F2'][dw][...],
                                     a1r[sl1][:, lo + dw:hi + dw],
                                     start=(dw == 0), stop=(dw == 5),
                                     skip_group_check=True)

            def f2_post(pair, j, t):
                ln = _wlen('f2', j)
                slot = t % 2
                nc.scalar.activation(a2r[slot][:, 0:ln], P_F2[0:104, 0:ln],
                                     AF.Sigmoid, bias=W['bF2'][...])

            def f3_half(pair, j, t, half):
                ln = _wlen('f3', j)
                hs = halves(ln)
                if half >= len(hs):
                    return
                lo, hi = hs[half]
                slot = t % 2
                t0 = STEP * j
                for dw in range(6):
                    nc.tensor.matmul(
                        P_F3[0:4, 0:hi - lo], W['F3'][dw][...],
                        a2r[slot][0:40, lo + dw:hi + dw],
                        start=(dw == 0), stop=(dw == 5),
                        tile_position=(0, 0), skip_group_check=True)
                    nc.tensor.matmul(
                        P_F3[64:68, 0:hi - lo], W['F3d'][dw][64:104],
                        a2r[slot][64:104, lo + dw:hi + dw],
                        start=(dw == 0), stop=(dw == 5),
                        tile_position=(64, 64), skip_group_check=True)
                nc.vector.tensor_scalar(of3[slot][:, 0:hi - lo],
                                        P_F3[0:68, 0:hi - lo],
                                        W['thr'][...], None, ALU.is_gt)
                for si in range(2):
                    s = pair * 2 + si
                    nc.gpsimd.dma_start(
                        out=out_d[s * 4:(s + 1) * 4, t0 + lo:t0 + hi],
                        in_=of3[slot][si * 64:si * 64 + 4, 0:hi - lo])

            # ---------------- skewed global pipeline ----------------
            NTOT = NPAIR * NSUP           # 36 global supertiles
            for s in (0, 1, 2, 3):
                prefetch_S(s, dup_dram=(s < 2))

            def stage_pt(g, skew):
                tt = g - skew
                if 0 <= tt < NTOT:
                    return tt, tt // NSUP, tt % NSUP
                return None, None, None

            for g in range(NTOT + 3):
                tD, pD, jD = stage_pt(g, 0)
                t1_, p1, j1 = stage_pt(g, 1)
                t2_, p2, j2 = stage_pt(g, 2)
                t3_, p3, j3 = stage_pt(g, 3)

                # S prefetch for the next pair, early in the pair
                if tD is not None and jD == 2 and pD < NPAIR - 1:
                    prefetch_S(2 * (pD + 1))
                    prefetch_S(2 * (pD + 1) + 1)

                sD0 = 2 * pD if tD is not None else None
                sD1 = sD0 + 1 if tD is not None else None
                slD0 = (tD % 2) if tD is not None else None
                slD1 = 2 + (tD % 2) if tD is not None else None
                sF0 = 2 * p1 if t1_ is not None else None
                sF1 = sF0 + 1 if t1_ is not None else None
                slF0 = (t1_ % 2) if t1_ is not None else None
                slF1 = 2 + (t1_ % 2) if t1_ is not None else None

                if tD is not None:
                    dil_mm(sD0, jD, 0)
                if t1_ is not None:
                    f1_mm(sF0, j1, slF0, 0)
                if tD is not None:
                    dil_post(sD0, jD, slD0, 0)
                    dil_mm(sD0, jD, 1)
                if t2_ is not None:
                    f2_mm(p2, j2, t2_, 0)
                if t1_ is not None:
                    f1_mm(sF0, j1, slF0, 1)
                    f1_post(sF0, j1, slF0)
                if tD is not None:
                    dil_post(sD0, jD, slD0, 1)
                if t3_ is not None:
                    f3_half(p3, j3, t3_, 0)
                if tD is not None:
                    dil_mm(sD1, jD, 0)
                if t2_ is not None:
                    f2_mm(p2, j2, t2_, 1)
                if t1_ is not None:
                    f1_mm(sF1, j1, slF1, 0)
                if tD is not None:
                    dil_post(sD1, jD, slD1, 0)
                    dil_mm(sD1, jD, 1)
                if t1_ is not None:
                    f1_mm(sF1, j1, slF1, 1)
                    f1_post(sF1, j1, slF1)
                if tD is not None:
                    dil_post(sD1, jD, slD1, 1)
                if t2_ is not None:
                    f2_post(p2, j2, t2_)
                if t3_ is not None:
                    f3_half(p3, j3, t3_, 1)

    return _patch_serialization(nc)


def kernel(**inputs):
    inputs = {k: np.asarray(v) for k, v in inputs.items()}
    in_maps = _shard_inputs(inputs)
    nc = build_bass()
    from concourse.bass_utils import run_bass_kernel_spmd
    res = run_bass_kernel_spmd(nc, in_maps, core_ids=list(range(NCORES)))
    outs = [res.results[i]['out'].reshape(BS, 4, T3) for i in range(NCORES)]
    full = np.concatenate(outs, axis=0)[:, None]  # [64, 1, 4, T3]
    return full.astype(np.float32)


# revision 28
# speedup vs baseline: 1.1842x; 1.1842x over previous
"""Trainium2 Bass kernel for nn_Allocator2 (dense_cnn), 8 NeuronCores.

Pure data parallelism: batch 64 -> 8 samples per core, weights replicated.

v2 redesign around PE array packing + a skewed windowed pipeline:

  head : 1x1 convs packed across 8 samples with block-diagonal weights,
         PSUM borrowed from the stage tiles.
  dil  : output in hh-major layout, two chunks with a duplicated row band:
           A = (hh 0..3) x 25 branches = 100 rows (r = 25*hh + i)
           B = (hh 3..6) x 25 branches = 100 rows
         row-packed: A at array rows 0..51, B at rows 64..115 (S and the
         B weights are duplicated at partitions 64+), so the two matmuls
         run concurrently in disjoint row groups.
  F1   : two chains, h in {0,1,2} on chunk A and h in {3,4,5} on chunk B,
         each K=100, M=48 (m = 16*h_loc + o).  The chains share the same
         lhsT (translation invariance in h) and run concurrently in col
         groups (out partitions 0..47 / 64..111), interleaved
         accumulation chains in one PSUM bank pair.
  F2   : K=112 over the gapped a1 layout (zero weights in rows 48..63),
         M=40 (m = 5*o + h), 2-sample col packing (out partitions 0..39
         and 64..103), one shared a2 pair tile + single ACT.
  F3   : baked K=120 (3 phases) x 2 taps, 2-sample col packing, single
         vector threshold per tile (round(sigmoid) == z > -bF3).

  Stage buffers are ~1KB ring slots stepped by STEP=1008 with cascaded
  extra columns (dil +15, F1 +10, F2 +5, bake +3) so each consumer tile j
  reads only producer slot j.  Global skewed pipeline over 36 (pair,
  supertile) ticks: tick g runs dil(g), F1(g-1), F2+bake(g-2), F3(g-3),
  so the PE never drains between stages, samples, or pairs.
"""

import numpy as np
import ml_dtypes

BF16 = ml_dtypes.bfloat16

B = 64            # global batch
NCORES = 8
BS = B // NCORES  # 8 samples per core
ND = 25
L = 8192          # concat length (4096 + 4096)
LX = 4096
LC = L - ND       # 8167 dilated output length
T1 = LC - 5       # 8162 F1 output length
T2 = T1 - 5       # 8157
T3 = T2 - 5       # 8152
NT = 512          # matmul free-dim tile
STEP = 1008       # supertile step
WIN = 1024        # ring slot width (2 PSUM banks of fp32)
NSUP = -(-LC // STEP)   # 9 supertiles per sample
NPAIR = BS // 2


def _bd(blocks):
    """block-diagonal stack of 2D arrays"""
    rs = sum(b.shape[0] for b in blocks)
    cs = sum(b.shape[1] for b in blocks)
    out = np.zeros((rs, cs), np.float32)
    r = c = 0
    for b in blocks:
        out[r:r + b.shape[0], c:c + b.shape[1]] = b
        r += b.shape[0]
        c += b.shape[1]
    return out


def build_weights(inp):
    """Host-side weight prep. Returns dict of np arrays (bf16 weights,
    fp32 biases) shared by all cores."""
    w = {}
    f32 = np.float32

    # ---- head: block-diagonal over BS samples, lhsT layout [K, M] ----
    def head_lhsT(wmat):  # wmat [Co, Ci] -> lhsT [Ci, Co] per sample
        return _bd([wmat.T.astype(f32)] * BS)

    w['hT1'] = head_lhsT(inp['wT1'])   # [72, 48]
    w['hT2'] = head_lhsT(inp['wT2'])   # [48, 32]
    w['hT3'] = head_lhsT(inp['wT3'])   # [32, 16]
    w['hR1'] = head_lhsT(inp['wR1'])   # [24, 16]
    w['hR2'] = head_lhsT(inp['wR2'])   # [16, 16]
    for nm in ('bT1', 'bT2', 'bT3', 'bR1', 'bR2'):
        w['hb' + nm[1:]] = np.tile(inp[nm].astype(f32), BS)[:, None]

    # ---- dil: original [52, 175], cols remapped to hh-major chunks ----
    dil = np.zeros((52, 175), f32)
    wM = inp['wM'].astype(f32)  # [25, 7, 2, 2]
    for i in range(ND):
        for o in range(7):
            m = i * 7 + o
            for c in range(2):
                dil[c * 26 + 0, m] = wM[i, o, c, 0]          # shift 0 tap
                dil[c * 26 + (i + 1), m] = wM[i, o, c, 1]    # shift i+1 tap
    dilA = np.zeros((52, 100), f32)   # cols 25*o + i, o in 0..3
    dilB = np.zeros((52, 100), f32)   # cols 25*(o-3) + i, o in 3..6
    bMA = np.zeros((100,), f32)
    bMB = np.zeros((100,), f32)
    for i in range(ND):
        for o in range(7):
            if o < 4:
                dilA[:, 25 * o + i] = dil[:, i * 7 + o]
                bMA[25 * o + i] = inp['bM'][i, o]
            if o >= 3:
                dilB[:, 25 * (o - 3) + i] = dil[:, i * 7 + o]
                bMB[25 * (o - 3) + i] = inp['bM'][i, o]
    w['dilA'] = dilA
    w['dilB'] = dilB
    w['bMA'] = bMA[:, None]
    w['bMB'] = bMB[:, None]

    # ---- F1: 6 lhsT [100, 48], shared by both chains ----
    wF1 = inp['wF1'].astype(f32)  # [16, 25, 2, 6]
    f1 = np.zeros((6, 100, 48), f32)
    for dw in range(6):
        for hl in range(3):
            for hhl in range(4):
                dh = hhl - hl
                if 0 <= dh <= 1:
                    for o in range(16):
                        for i in range(ND):
                            f1[dw, 25 * hhl + i, 16 * hl + o] = wF1[o, i, dh, dw]
    w['F1'] = f1
    # bias in a1 layout [112]: h<3 -> 16h+o ; h>=3 -> 64+16(h-3)+o
    bF1 = np.zeros((112,), f32)
    for h in range(6):
        r0 = 16 * h if h < 3 else 64 + 16 * (h - 3)
        for o in range(16):
            bF1[r0 + o] = inp['bF1'][o]
    w['bF1'] = bF1[:, None]

    # ---- F2: 6 lhsT [112, 40] over gapped a1 rows, m = 5*o + h ----
    wF2 = inp['wF2'].astype(f32)  # [8, 16, 2, 6]
    f2 = np.zeros((6, 112, 40), f32)
    for dw in range(6):
        for hh in range(6):
            r0 = 16 * hh if hh < 3 else 64 + 16 * (hh - 3)
            for ci in range(16):
                for o in range(8):
                    for h in range(5):
                        dh = hh - h
                        if 0 <= dh <= 1:
                            f2[dw, r0 + ci, o * 5 + h] = wF2[o, ci, dh, dw]
    w['F2'] = f2
    bF2 = np.zeros((104,), f32)
    for o in range(8):
        for h in range(5):
            bF2[o * 5 + h] = inp['bF2'][o]
            bF2[64 + o * 5 + h] = inp['bF2'][o]
    w['bF2'] = bF2[:, None]

    # ---- F3 direct: lhsT[dw] [40, 4]; row m = ci*5 + hh, col h ----
    wF3 = inp['wF3'].astype(f32)  # [1, 8, 2, 6]
    f3 = np.zeros((6, 40, 4), f32)
    for dw in range(6):
        for ci in range(8):
            for hh in range(5):
                for h in range(4):
                    dh = hh - h
                    if 0 <= dh <= 1:
                        f3[dw, ci * 5 + hh, h] = wF3[0, ci, dh, dw]
    w['F3'] = f3
    thr = np.zeros((68,), f32)
    thr[0:4] = -inp['bF3'][0]
    thr[64:68] = -inp['bF3'][0]
    w['thr'] = thr[:, None]

    # bf16-ify matmul weights
    for k in ('hT1', 'hT2', 'hT3', 'hR1', 'hR2', 'dilA', 'dilB',
              'F1', 'F2', 'F3'):
        w[k] = w[k].astype(BF16)
    return w


def emulate_core(w, x_core, y_core):
    """Numpy emulation of exactly what the Bass kernel computes for one
    core. x_core [72, 4096] bf16, y_core [24, 4096] bf16. Returns
    [BS, 4, T3] f32 in {0,1}."""
    f32 = np.float32

    def mm(lhsT, rhs):  # bf16 operands, f32 accumulate
        return lhsT.astype(f32).T @ rhs.astype(f32)

    relu = lambda a: np.maximum(a, 0)
    sig = lambda a: 1.0 / (1.0 + np.exp(-a))

    a = relu(mm(w['hT1'], x_core) + w['hbT1']).astype(BF16)
    a = relu(mm(w['hT2'], a) + w['hbT2']).astype(BF16)
    t3 = (mm(w['hT3'], a) + w['hbT3']).astype(BF16)          # [16, 4096]
    b_ = relu(mm(w['hR1'], y_core) + w['hbR1']).astype(BF16)
    b_ = relu(mm(w['hR2'], b_) + w['hbR2']).astype(BF16)     # [16, 4096]
    out2 = np.concatenate([t3, b_], axis=1)                  # [16, 8192] bf16

    res = np.zeros((BS, 4, T3), f32)
    for s in range(BS):
        o2 = out2[s * 2:s * 2 + 2]                           # [2, 8192]
        S = np.zeros((52, LC), BF16)
        for c in range(2):
            for sh in range(26):
                S[c * 26 + sh] = o2[c, sh:sh + LC]
        OA = relu(mm(w['dilA'], S) + w['bMA']).astype(BF16)   # [100, LC]
        OB = relu(mm(w['dilB'], S) + w['bMB']).astype(BF16)   # [100, LC]
        z1 = np.zeros((112, T1), f32)
        for dw in range(6):
            z1[0:48] += mm(w['F1'][dw], OA[:, dw:dw + T1])
            z1[64:112] += mm(w['F1'][dw], OB[:, dw:dw + T1])
        a1 = sig(z1 + w['bF1']).astype(BF16)                 # [112, T1]
        a1[48:64] = 0.0                                      # gap rows
        z2 = np.zeros((40, T2), f32)
        for dw in range(6):
            z2 += mm(w['F2'][dw], a1[:, dw:dw + T2])
        a2 = sig(z2 + w['bF2'][0:40]).astype(BF16)           # [40, T2]
        z3 = np.zeros((4, T3), f32)
        for dw in range(6):
            z3 += mm(w['F3'][dw], a2[:, dw:dw + T3])
        res[s] = (z3 > w['thr'][0:4]).astype(f32)            # [4, T3]
    return res


def _shard_inputs(inputs):
    """Build per-core in_maps (host-side prep + shard)."""
    w = build_weights(inputs)
    in_maps = []
    for c in range(NCORES):
        m = dict(w)
        xs = inputs['x'][c * BS:(c + 1) * BS]  # [8, 9, 4096]
        ys = inputs['y'][c * BS:(c + 1) * BS]
        m['x'] = np.ascontiguousarray(xs.reshape(BS * 9, LX)).astype(BF16)
        m['y'] = np.ascontiguousarray(ys.reshape(BS * 3, LX)).astype(BF16)
        in_maps.append(m)
    return in_maps


# ---------------------------------------------------------------------------
# Bass program
# ---------------------------------------------------------------------------

def _split_excess_waits(bir, maxw=1):
    """The walrus build in this container refuses instructions carrying
    more than ~1 semaphore wait ("Too many sync wait commands").  Tile
    attaches multi-waits freely.  Splitting is semantics-preserving: move
    excess waits onto injected NoOps on the same engine immediately
    before the instruction (engines execute their instruction stream in
    order, so wait-all is preserved)."""
    for fn in bir['functions']:
        for bb in fn['blocks']:
            out = []
            for inst in bb['instructions']:
                si = inst.get('sync_info')
                waits = (si or {}).get('on_wait') or []
                if len(waits) > maxw:
                    extra, keep = waits[:-maxw], waits[-maxw:]
                    for i in range(0, len(extra), maxw):
                        out.append({
                            "debug": inst.get("debug", 0),
                            "engine": inst["engine"], "ins": [],
                            "name": f"{inst['name']}-wsplit{i}",
                            "opcode": "NoOp", "outs": [],
                            "sync_info": {"on_update": [],
                                          "on_wait": extra[i:i + maxw]}})
                    si['on_wait'] = keep
                out.append(inst)
            bb['instructions'] = out
    return bir


def _patch_serialization(nc):
    import orjson
    bir = _split_excess_waits(nc.to_json())
    patched = orjson.dumps(bir)
    nc.to_json_bytes = lambda: patched
    return nc


# per-stage output lengths and window extensions
EXT = {'dil': 15, 'f1': 10, 'f2': 5, 'f3': 0}
LST = {'dil': LC, 'f1': T1, 'f2': T2, 'f3': T3}


def _wlen(stage, j):
    """columns stage tile j computes: [STEP*j, STEP*j + wlen)"""
    t0 = STEP * j
    if t0 >= LST[stage]:
        return 0
    return min(t0 + STEP + EXT[stage], LST[stage]) - t0


def build_bass():
    import bass_rust
    import concourse.bass as bass
    import concourse.mybir as mybir
    from concourse.tile import TileContext

    dt = mybir.dt
    AF = mybir.ActivationFunctionType
    ALU = mybir.AluOpType

    nc = bass.Bass()

    p = {}
    p['x'] = nc.declare_dram_parameter('x', [BS * 9, LX], dt.bfloat16, False)
    p['y'] = nc.declare_dram_parameter('y', [BS * 3, LX], dt.bfloat16, False)
    for nm, sh in [('hT1', [BS * 9, BS * 6]), ('hT2', [BS * 6, BS * 4]),
                   ('hT3', [BS * 4, BS * 2]),
                   ('hR1', [BS * 3, BS * 2]), ('hR2', [BS * 2, BS * 2]),
                   ('dilA', [52, 100]), ('dilB', [52, 100]),
                   ('F1', [6, 100, 48]), ('F2', [6, 112, 40]),
                   ('F3', [6, 40, 4])]:
        p[nm] = nc.declare_dram_parameter(nm, sh, dt.bfloat16, False)
    for nm, sh in [('hbT1', [BS * 6, 1]), ('hbT2', [BS * 4, 1]),
                   ('hbT3', [BS * 2, 1]),
                   ('hbR1', [BS * 2, 1]), ('hbR2', [BS * 2, 1]),
                   ('bMA', [100, 1]), ('bMB', [100, 1]),
                   ('bF1', [112, 1]), ('bF2', [104, 1]), ('thr', [68, 1])]:
        p[nm] = nc.declare_dram_parameter(nm, sh, dt.float32, False)
    out_d = nc.declare_dram_parameter('out', [BS * 4, T3], dt.bfloat16, True)
    # DRAM staging for o2t: window reads from HBM avoid the SBUF
    # per-partition read bottleneck (26x overlapping reads of one row)
    o2d = nc.dram_tensor('o2d', [BS * 2, L], dt.bfloat16)

    with TileContext(nc) as tc:
        with tc.tile_pool(name="wpool", bufs=1) as wp, \
             tc.tile_pool(name="head", bufs=1) as hp, \
             tc.tile_pool(name="ring", bufs=1) as rp, \
             tc.tile_pool(name="psum", bufs=1, space="PSUM") as pp:

            # ---------------- weights to SBUF ----------------
            # head weights + inputs on the sync queue (head critical path);
            # stage weights on gpsimd (hidden under the head)
            xt = hp.tile([BS * 9, LX], dt.bfloat16, name="xt")
            yt = hp.tile([BS * 3, LX], dt.bfloat16, name="yt")
            nc.sync.dma_start(out=xt[...], in_=p['x'][...])
            W = {}
            for nm in ('hT1', 'hbT1', 'hR1', 'hbR1', 'hT2', 'hbT2',
                       'hR2', 'hbR2', 'hT3', 'hbT3'):
                t = wp.tile(list(p[nm].shape), p[nm].dtype, name=f"w_{nm}")
                nc.sync.dma_start(out=t[...], in_=p[nm][...])
                W[nm] = t
            for nm in ('dilA', 'bMA', 'bMB', 'bF1', 'bF2', 'thr'):
                t = wp.tile(list(p[nm].shape), p[nm].dtype, name=f"w_{nm}")
                nc.gpsimd.dma_start(out=t[...], in_=p[nm][...])
                W[nm] = t
            # dilB duplicated at partitions 64..115 for row tiling
            w_dilB = wp.tile([116, 100], dt.bfloat16, name="w_dilB")
            nc.gpsimd.dma_start(out=w_dilB[64:116], in_=p['dilB'][...])
            W['dilB'] = w_dilB
            for nm in ('F1', 'F2', 'F3'):
                n_sl, kk, mm_ = p[nm].shape
                W[nm] = []
                for i_sl in range(n_sl):
                    t = wp.tile([kk, mm_], p[nm].dtype, name=f"w_{nm}{i_sl}")
                    nc.gpsimd.dma_start(out=t[...], in_=p[nm][i_sl])
                    W[nm].append(t)
            # F3 weights duplicated at partitions 64..103 for row tiling
            W['F3d'] = []
            for i_sl in range(6):
                t = wp.tile([104, 4], dt.bfloat16, name=f"w_F3d{i_sl}")
                nc.gpsimd.dma_start(out=t[64:104], in_=p['F3'][i_sl])
                W['F3d'].append(t)

            # ---------------- static PSUM tiles (8 banks) ----------------
            P_DA = pp.tile([128, WIN], dt.float32, name="P_DA")    # 2 banks
            P_DB = pp.tile([128, NT], dt.float32, name="P_DB")     # 1 bank
            P_F1 = pp.tile([128, WIN], dt.float32, name="P_F1")    # 2 banks
            P_F2 = pp.tile([128, WIN], dt.float32, name="P_F2")    # 2 banks
            P_F3 = pp.tile([128, NT], dt.float32, name="P_F3")     # 1 bank

            # one-time zero of the F1 gap rows (never matmul-written)
            nc.vector.memset(P_F1[32:64, :], 0.0)

            # ---------------- head ----------------
            nc.sync.dma_start(out=yt[...], in_=p['y'][...])
            o2t = hp.tile([BS * 2, L], dt.bfloat16, name="o2t")
            head_ps = [P_DA, P_F1, P_F2]

            def head_layer(w_nm, b_nm, rows_in, rows_out, src, dst, act,
                           dst_off=0):
                for j in range(LX // WIN):
                    pt = head_ps[j % 3]
                    for h in range(2):
                        sl = slice(j * WIN + h * NT, j * WIN + (h + 1) * NT)
                        nc.tensor.matmul(pt[0:rows_out, h * NT:(h + 1) * NT],
                                         W[w_nm][...], src[:rows_in, sl],
                                         start=True, stop=True)
                    sl2 = slice(dst_off + j * WIN, dst_off + (j + 1) * WIN)
                    ps = pt[0:rows_out, 0:WIN]
                    if act == 'relu':
                        nc.scalar.activation(dst[:rows_out, sl2], ps,
                                             AF.Relu, bias=W[b_nm][...])
                    else:
                        nc.vector.tensor_scalar(dst[:rows_out, sl2], ps,
                                                W[b_nm][...], None, ALU.add)

            a1h = hp.tile([BS * 6, LX], dt.bfloat16, name="a1h")
            a2h = hp.tile([BS * 6, LX], dt.bfloat16, name="a2h")
            b1h = hp.tile([BS * 6, LX], dt.bfloat16, name="b1h")
            head_layer('hT1', 'hbT1', BS * 9, BS * 6, xt, a1h, 'relu')
            head_layer('hT2', 'hbT2', BS * 6, BS * 4, a1h, a2h, 'relu')
            head_layer('hT3', 'hbT3', BS * 4, BS * 2, a2h, o2t, 'add')
            head_layer('hR1', 'hbR1', BS * 3, BS * 2, yt, b1h, 'relu')
            head_layer('hR2', 'hbR2', BS * 2, BS * 2, b1h, o2t, 'relu',
                       dst_off=LX)
            # stage o2t to DRAM for the window builds
            nc.sync.dma_start(out=o2d[...], in_=o2t[...])

            # ---------------- ring buffers ----------------
            S_bufs = [rp.tile([116, LC], dt.bfloat16, name=f"S{i}")
                      for i in range(4)]
            # per-sample rings: slot = 2*(s%2) + (t%2), t = 9*pair + j
            Oa = [rp.tile([100, WIN], dt.bfloat16, name=f"Oa{i}")
                  for i in range(4)]
            Ob = [rp.tile([100, WIN], dt.bfloat16, name=f"Ob{i}")
                  for i in range(4)]
            a1r = [rp.tile([112, WIN], dt.bfloat16, name=f"a1r{i}")
                   for i in range(4)]
            # per-pair rings: slot = t%2
            a2r = [rp.tile([104, WIN], dt.bfloat16, name=f"a2r{i}")
                   for i in range(2)]
            of3 = [rp.tile([68, NT], dt.bfloat16, name=f"of3{i}")
                   for i in range(2)]

            def prefetch_S(s, dup_dram=True):
                """window DMAs (from DRAM staging) building S for sample s,
                primary rows c*26+w plus the row-64 duplicate."""
                St = S_bufs[s % 4]
                qs = [nc.scalar, nc.sync]
                for c in range(2):
                    for dd in range(2):
                        win = o2d[s * 2 + c:s * 2 + c + 1, 0:LC].copy()
                        win.ap = bass_rust.VecI64Pair(
                            [[L, 1], [1, 26], [1, LC]])
                        r0 = 64 * dd + c * 26
                        qs[(c + dd) % 2].dma_start(
                            out=St[r0:r0 + 26, :], in_=win)

            def halves(ln):
                out = [(0, min(NT, ln))]
                if ln > NT:
                    out.append((NT, ln))
                return out

            # ---------------- stage emitters ----------------
            def dil_mm(s, j, half):
                ln = _wlen('dil', j)
                hs = halves(ln)
                if half >= len(hs):
                    return
                lo, hi = hs[half]
                St = S_bufs[s % 4]
                t0 = STEP * j
                nc.tensor.matmul(P_DA[0:100, lo:hi], W['dilA'][...],
                                 St[0:52, t0 + lo:t0 + hi],
                                 start=True, stop=True, tile_position=(0, 0),
                                 skip_group_check=True)
                nc.tensor.matmul(P_DB[0:100, 0:hi - lo], W['dilB'][64:116],
                                 St[64:116, t0 + lo:t0 + hi],
                                 start=True, stop=True, tile_position=(64, 0),
                                 skip_group_check=True)

            def dil_post(s, j, slot, half):
                ln = _wlen('dil', j)
                hs = halves(ln)
                if half >= len(hs):
                    return
                lo, hi = hs[half]
                nc.vector.tensor_scalar(Ob[slot][:, lo:hi],
                                        P_DB[0:100, 0:hi - lo],
                                        W['bMB'][...], 0.0, ALU.add, ALU.max)
                if half == len(hs) - 1:
                    nc.scalar.activation(Oa[slot][:, 0:ln], P_DA[0:100, 0:ln],
                                         AF.Relu, bias=W['bMA'][...])

            def f1_mm(s, j, slot, half):
                ln = _wlen('f1', j)
                hs = halves(ln)
                if half >= len(hs):
                    return
                lo, hi = hs[half]
                for dw in range(6):
                    nc.tensor.matmul(P_F1[0:48, lo:hi], W['F1'][dw][...],
                                     Oa[slot][:, lo + dw:hi + dw],
                                     start=(dw == 0), stop=(dw == 5),
                                     skip_group_check=True)
                    nc.tensor.matmul(P_F1[64:112, lo:hi], W['F1'][dw][...],
                                     Ob[slot][:, lo + dw:hi + dw],
                                     start=(dw == 0), stop=(dw == 5),
                                     skip_group_check=True)

            def f1_post(s, j, slot):
                ln = _wlen('f1', j)
                nc.scalar.activation(a1r[slot][:, 0:ln], P_F1[0:112, 0:ln],
                                     AF.Sigmoid, bias=W['bF1'][...])

            def f2_mm(pair, j, t, half):
                ln = _wlen('f2', j)
                hs = halves(ln)
                if half >= len(hs):
                    return
                lo, hi = hs[half]
                sl0 = 2 * 0 + (t % 2)   # sample 2*pair
                sl1 = 2 * 1 + (t % 2)   # sample 2*pair+1
                for dw in range(6):
                    nc.tensor.matmul(P_F2[0:40, lo:hi], W['F2'][dw][...],
                                     a1r[sl0][:, lo + dw:hi + dw],
                                     start=(dw == 0), stop=(dw == 5),
                                     skip_group_check=True)
                    nc.tensor.matmul(P_F2[64:104, lo:hi], W['F2'][dw][...],
                                     a1r[sl1][:, lo + dw:hi + dw],
                                     start=(dw == 0), stop=(dw == 5),
                                     skip_group_check=True)

            def f2_post(pair, j, t):
                ln = _wlen('f2', j)
                slot = t % 2
                nc.scalar.activation(a2r[slot][:, 0:ln], P_F2[0:104, 0:ln],
                                     AF.Sigmoid, bias=W['bF2'][...])

            def f3_half(pair, j, t, half):
                ln = _wlen('f3', j)
                hs = halves(ln)
                if half >= len(hs):
                    return
                lo, hi = hs[half]
                slot = t % 2
                t0 = STEP * j
                for dw in range(6):
                    nc.tensor.matmul(
                        P_F3[0:4, 0:hi - lo], W['F3'][dw][...],
                        a2r[slot][0:40, lo + dw:hi + dw],
                        start=(dw == 0), stop=(dw == 5),
                        tile_position=(0, 0), skip_group_check=True)
                    nc.tensor.matmul(
                        P_F3[64:68, 0:hi - lo], W['F3d'][dw][64:104],
                        a2r[slot][64:104, lo + dw:hi + dw],
                        start=(dw == 0), stop=(dw == 5),
                        tile_position=(64, 64), skip_group_check=True)
                nc.vector.tensor_scalar(of3[slot][:, 0:hi - lo],
                                        P_F3[0:68, 0:hi - lo],
                                        W['thr'][...], None, ALU.is_gt)
                for si in range(2):
                    s = pair * 2 + si
                    nc.gpsimd.dma_start(
                        out=out_d[s * 4:(s + 1) * 4, t0 + lo:t0 + hi],
                        in_=of3[slot][si * 64:si * 64 + 4, 0:hi - lo])

            # ---------------- skewed global pipeline ----------------
            NTOT = NPAIR * NSUP           # 36 global supertiles
            for s in (0, 1, 2, 3):
                prefetch_S(s, dup_dram=(s < 2))

            def stage_pt(g, skew):
                tt = g - skew
                if 0 <= tt < NTOT:
                    return tt, tt // NSUP, tt % NSUP
                return None, None, None

            for g in range(NTOT + 3):
                tD, pD, jD = stage_pt(g, 0)
                t1_, p1, j1 = stage_pt(g, 1)
                t2_, p2, j2 = stage_pt(g, 2)
                t3_, p3, j3 = stage_pt(g, 3)

                # S prefetch for the next pair, early in the pair
                if tD is not None and jD == 2 and pD < NPAIR - 1:
                    prefetch_S(2 * (pD + 1))
                    prefetch_S(2 * (pD + 1) + 1)

                sD0 = 2 * pD if tD is not None else None
                sD1 = sD0 + 1 if tD is not None else None
                slD0 = (tD % 2) if tD is not None else None
                slD1 = 2 + (tD % 2) if tD is not None else None
                sF0 = 2 * p1 if t1_ is not None else None
                sF1 = sF0 + 1 if t1_ is not None else None
                slF0 = (t1_ % 2) if t1_ is not None else None
                slF1 = 2 + (t1_ % 2) if t1_ is not None else None

                if tD is not None:
                    dil_mm(sD0, jD, 0)
                if t1_ is not None:
                    f1_mm(sF0, j1, slF0, 0)
                if tD is not None:
                    dil_post(sD0, jD, slD0, 0)
                    dil_mm(sD0, jD, 1)
                if t2_ is not None:
                    f2_mm(p2, j2, t2_, 0)
                if t1_ is not None:
                    f1_mm(sF0, j1, slF0, 1)
                    f1_post(sF0, j1, slF0)
                if tD is not None:
                    dil_post(sD0, jD, slD0, 1)
                if t3_ is not None:
                    f3_half(p3, j3, t3_, 0)
                if tD is not None:
                    dil_mm(sD1, jD, 0)
                if t2_ is not None:
                    f2_mm(p2, j2, t2_, 1)
                if t1_ is not None:
                    f1_mm(sF1, j1, slF1, 0)
                if tD is not None:
                    dil_post(sD1, jD, slD1, 0)
                    dil_mm(sD1, jD, 1)
                if t1_ is not None:
                    f1_mm(sF1, j1, slF1, 1)
                    f1_post(sF1, j1, slF1)
                if tD is not None:
                    dil_post(sD1, jD, slD1, 1)
                if t2_ is not None:
                    f2_post(p2, j2, t2_)
                if t3_ is not None:
                    f3_half(p3, j3, t3_, 1)

    return _patch_serialization(nc)


def kernel(**inputs):
    inputs = {k: np.asarray(v) for k, v in inputs.items()}
    in_maps = _shard_inputs(inputs)
    nc = build_bass()
    from concourse.bass_utils import run_bass_kernel_spmd
    res = run_bass_kernel_spmd(nc, in_maps, core_ids=list(range(NCORES)))
    outs = [res.results[i]['out'].reshape(BS, 4, T3) for i in range(NCORES)]
    full = np.concatenate(outs, axis=0)[:, None]  # [64, 1, 4, T3]
    return full.astype(np.float32)


# revision 42
# speedup vs baseline: 1.2891x; 1.0886x over previous
"""Trainium2 Bass kernel for nn_Allocator2 (dense_cnn), 8 NeuronCores.

Pure data parallelism: batch 64 -> 8 samples per core, weights replicated.

v2 redesign around PE array packing + a skewed windowed pipeline:

  head : 1x1 convs packed across 8 samples with block-diagonal weights,
         PSUM borrowed from the stage tiles.
  dil  : output in hh-major layout, two chunks with a duplicated row band:
           A = (hh 0..3) x 25 branches = 100 rows (r = 25*hh + i)
           B = (hh 3..6) x 25 branches = 100 rows
         row-packed: A at array rows 0..51, B at rows 64..115 (S and the
         B weights are duplicated at partitions 64+), so the two matmuls
         run concurrently in disjoint row groups.
  F1   : two chains, h in {0,1,2} on chunk A and h in {3,4,5} on chunk B,
         each K=100, M=48 (m = 16*h_loc + o).  The chains share the same
         lhsT (translation invariance in h) and run concurrently in col
         groups (out partitions 0..47 / 64..111), interleaved
         accumulation chains in one PSUM bank pair.
  F2   : K=112 over the gapped a1 layout (zero weights in rows 48..63),
         M=40 (m = 5*o + h), 2-sample col packing (out partitions 0..39
         and 64..103), one shared a2 pair tile + single ACT.
  F3   : baked K=120 (3 phases) x 2 taps, 2-sample col packing, single
         vector threshold per tile (round(sigmoid) == z > -bF3).

  Stage buffers are ~1KB ring slots stepped by STEP=1008 with cascaded
  extra columns (dil +15, F1 +10, F2 +5, bake +3) so each consumer tile j
  reads only producer slot j.  Global skewed pipeline over 36 (pair,
  supertile) ticks: tick g runs dil(g), F1(g-1), F2+bake(g-2), F3(g-3),
  so the PE never drains between stages, samples, or pairs.
"""

import numpy as np
import ml_dtypes

BF16 = ml_dtypes.bfloat16

B = 64            # global batch
NCORES = 8
BS = B // NCORES  # 8 samples per core
ND = 25
L = 8192          # concat length (4096 + 4096)
LX = 4096
LC = L - ND       # 8167 dilated output length
T1 = LC - 5       # 8162 F1 output length
T2 = T1 - 5       # 8157
T3 = T2 - 5       # 8152
NT = 512          # matmul free-dim tile
STEP = 1008       # supertile step
WIN = 1024        # ring slot width (2 PSUM banks of fp32)
NSUP = -(-LC // STEP)   # 9 supertiles per sample
NPAIR = BS // 2


def _bd(blocks):
    """block-diagonal stack of 2D arrays"""
    rs = sum(b.shape[0] for b in blocks)
    cs = sum(b.shape[1] for b in blocks)
    out = np.zeros((rs, cs), np.float32)
    r = c = 0
    for b in blocks:
        out[r:r + b.shape[0], c:c + b.shape[1]] = b
        r += b.shape[0]
        c += b.shape[1]
    return out


def build_weights(inp):
    """Host-side weight prep. Returns dict of np arrays (bf16 weights,
    fp32 biases) shared by all cores."""
    w = {}
    f32 = np.float32

    # ---- head: block-diagonal over BS samples, lhsT layout [K, M] ----
    def head_lhsT(wmat):  # wmat [Co, Ci] -> lhsT [Ci, Co] per sample
        return _bd([wmat.T.astype(f32)] * BS)

    w['hT1'] = head_lhsT(inp['wT1'])   # [72, 48]
    w['hT2'] = head_lhsT(inp['wT2'])   # [48, 32]
    w['hT3'] = head_lhsT(inp['wT3'])   # [32, 16]
    w['hR1'] = head_lhsT(inp['wR1'])   # [24, 16]
    w['hR2'] = head_lhsT(inp['wR2'])   # [16, 16]
    for nm in ('bT1', 'bT2', 'bT3', 'bR1', 'bR2'):
        w['hb' + nm[1:]] = np.tile(inp[nm].astype(f32), BS)[:, None]

    # ---- dil: original [52, 175], cols remapped to hh-major chunks ----
    dil = np.zeros((52, 175), f32)
    wM = inp['wM'].astype(f32)  # [25, 7, 2, 2]
    for i in range(ND):
        for o in range(7):
            m = i * 7 + o
            for c in range(2):
                dil[c * 26 + 0, m] = wM[i, o, c, 0]          # shift 0 tap
                dil[c * 26 + (i + 1), m] = wM[i, o, c, 1]    # shift i+1 tap
    dilA = np.zeros((52, 100), f32)   # cols 25*o + i, o in 0..3
    dilB = np.zeros((52, 100), f32)   # cols 25*(o-3) + i, o in 3..6
    bMA = np.zeros((100,), f32)
    bMB = np.zeros((100,), f32)
    for i in range(ND):
        for o in range(7):
            if o < 4:
                dilA[:, 25 * o + i] = dil[:, i * 7 + o]
                bMA[25 * o + i] = inp['bM'][i, o]
            if o >= 3:
                dilB[:, 25 * (o - 3) + i] = dil[:, i * 7 + o]
                bMB[25 * (o - 3) + i] = inp['bM'][i, o]
    w['dilA'] = dilA
    w['dilB'] = dilB
    w['bMA'] = bMA[:, None]
    w['bMB'] = bMB[:, None]

    # ---- F1: 6 lhsT [100, 48], shared by both chains ----
    wF1 = inp['wF1'].astype(f32)  # [16, 25, 2, 6]
    f1 = np.zeros((6, 100, 48), f32)
    for dw in range(6):
        for hl in range(3):
            for hhl in range(4):
                dh = hhl - hl
                if 0 <= dh <= 1:
                    for o in range(16):
                        for i in range(ND):
                            f1[dw, 25 * hhl + i, 16 * hl + o] = wF1[o, i, dh, dw]
    w['F1'] = f1
    # bias in a1 layout [112]: h<3 -> 16h+o ; h>=3 -> 64+16(h-3)+o
    bF1 = np.zeros((112,), f32)
    for h in range(6):
        r0 = 16 * h if h < 3 else 64 + 16 * (h - 3)
        for o in range(16):
            bF1[r0 + o] = inp['bF1'][o]
    w['bF1'] = bF1[:, None]

    # ---- F2: 6 lhsT [112, 40] over gapped a1 rows, m = 5*o + h ----
    wF2 = inp['wF2'].astype(f32)  # [8, 16, 2, 6]
    f2 = np.zeros((6, 112, 40), f32)
    for dw in range(6):
        for hh in range(6):
            r0 = 16 * hh if hh < 3 else 64 + 16 * (hh - 3)
            for ci in range(16):
                for o in range(8):
                    for h in range(5):
                        dh = hh - h
                        if 0 <= dh <= 1:
                            f2[dw, r0 + ci, o * 5 + h] = wF2[o, ci, dh, dw]
    w['F2'] = f2
    bF2 = np.zeros((104,), f32)
    for o in range(8):
        for h in range(5):
            bF2[o * 5 + h] = inp['bF2'][o]
            bF2[64 + o * 5 + h] = inp['bF2'][o]
    w['bF2'] = bF2[:, None]

    # ---- F3 direct: lhsT[dw] [40, 4]; row m = ci*5 + hh, col h ----
    # dw 3..5 negated: the fused threshold computes (pA - thr) > nB with
    # nB = -sum_{dw>=3} W.T a2, i.e. pA + pB > thr.
    wF3 = inp['wF3'].astype(f32)  # [1, 8, 2, 6]
    f3 = np.zeros((6, 40, 4), f32)
    for dw in range(6):
        sgn = 1.0 if dw < 3 else -1.0
        for ci in range(8):
            for hh in range(5):
                for h in range(4):
                    dh = hh - h
                    if 0 <= dh <= 1:
                        f3[dw, ci * 5 + hh, h] = sgn * wF3[0, ci, dh, dw]
    w['F3'] = f3
    thr = np.zeros((36,), f32)
    thr[0:4] = -inp['bF3'][0]
    thr[32:36] = -inp['bF3'][0]
    w['thr'] = thr[:, None]

    # bf16-ify matmul weights
    for k in ('hT1', 'hT2', 'hT3', 'hR1', 'hR2', 'dilA', 'dilB',
              'F1', 'F2', 'F3'):
        w[k] = w[k].astype(BF16)
    return w


def emulate_core(w, x_core, y_core):
    """Numpy emulation of exactly what the Bass kernel computes for one
    core. x_core [72, 4096] bf16, y_core [24, 4096] bf16. Returns
    [BS, 4, T3] f32 in {0,1}."""
    f32 = np.float32

    def mm(lhsT, rhs):  # bf16 operands, f32 accumulate
        return lhsT.astype(f32).T @ rhs.astype(f32)

    relu = lambda a: np.maximum(a, 0)
    sig = lambda a: 1.0 / (1.0 + np.exp(-a))

    a = relu(mm(w['hT1'], x_core) + w['hbT1']).astype(BF16)
    a = relu(mm(w['hT2'], a) + w['hbT2']).astype(BF16)
    t3 = (mm(w['hT3'], a) + w['hbT3']).astype(BF16)          # [16, 4096]
    b_ = relu(mm(w['hR1'], y_core) + w['hbR1']).astype(BF16)
    b_ = relu(mm(w['hR2'], b_) + w['hbR2']).astype(BF16)     # [16, 4096]
    out2 = np.concatenate([t3, b_], axis=1)                  # [16, 8192] bf16

    res = np.zeros((BS, 4, T3), f32)
    for s in range(BS):
        o2 = out2[s * 2:s * 2 + 2]                           # [2, 8192]
        S = np.zeros((52, LC), BF16)
        for c in range(2):
            for sh in range(26):
                S[c * 26 + sh] = o2[c, sh:sh + LC]
        OA = relu(mm(w['dilA'], S) + w['bMA']).astype(BF16)   # [100, LC]
        OB = relu(mm(w['dilB'], S) + w['bMB']).astype(BF16)   # [100, LC]
        z1 = np.zeros((112, T1), f32)
        for dw in range(6):
            z1[0:48] += mm(w['F1'][dw], OA[:, dw:dw + T1])
            z1[64:112] += mm(w['F1'][dw], OB[:, dw:dw + T1])
        a1 = sig(z1 + w['bF1']).astype(BF16)                 # [112, T1]
        a1[48:64] = 0.0                                      # gap rows
        z2 = np.zeros((40, T2), f32)
        for dw in range(6):
            z2 += mm(w['F2'][dw], a1[:, dw:dw + T2])
        a2 = sig(z2 + w['bF2'][0:40]).astype(BF16)           # [40, T2]
        pA = np.zeros((4, T3), f32)
        nB = np.zeros((4, T3), f32)
        for dw in range(3):
            pA += mm(w['F3'][dw], a2[:, dw:dw + T3])
        for dw in range(3, 6):
            nB += mm(w['F3'][dw], a2[:, dw:dw + T3])
        res[s] = ((pA - w['thr'][0:4]) > nB).astype(f32)     # [4, T3]
    return res


def _shard_inputs(inputs):
    """Build per-core in_maps (host-side prep + shard)."""
    w = build_weights(inputs)
    in_maps = []
    for c in range(NCORES):
        m = dict(w)
        xs = inputs['x'][c * BS:(c + 1) * BS]  # [8, 9, 4096]
        ys = inputs['y'][c * BS:(c + 1) * BS]
        m['x'] = np.ascontiguousarray(xs.reshape(BS * 9, LX)).astype(BF16)
        m['y'] = np.ascontiguousarray(ys.reshape(BS * 3, LX)).astype(BF16)
        in_maps.append(m)
    return in_maps


# ---------------------------------------------------------------------------
# Bass program
# ---------------------------------------------------------------------------

def _split_excess_waits(bir, maxw=1):
    """The walrus build in this container refuses instructions carrying
    more than ~1 semaphore wait ("Too many sync wait commands").  Tile
    attaches multi-waits freely.  Splitting is semantics-preserving: move
    excess waits onto injected NoOps on the same engine immediately
    before the instruction (engines execute their instruction stream in
    order, so wait-all is preserved)."""
    for fn in bir['functions']:
        for bb in fn['blocks']:
            out = []
            for inst in bb['instructions']:
                si = inst.get('sync_info')
                waits = (si or {}).get('on_wait') or []
                if len(waits) > maxw:
                    extra, keep = waits[:-maxw], waits[-maxw:]
                    for i in range(0, len(extra), maxw):
                        out.append({
                            "debug": inst.get("debug", 0),
                            "engine": inst["engine"], "ins": [],
                            "name": f"{inst['name']}-wsplit{i}",
                            "opcode": "NoOp", "outs": [],
                            "sync_info": {"on_update": [],
                                          "on_wait": extra[i:i + maxw]}})
                    si['on_wait'] = keep
                out.append(inst)
            bb['instructions'] = out
    return bir


def _patch_serialization(nc):
    import orjson
    bir = _split_excess_waits(nc.to_json())
    patched = orjson.dumps(bir)
    nc.to_json_bytes = lambda: patched
    return nc


# per-stage output lengths and window extensions
EXT = {'dil': 15, 'f1': 10, 'f2': 5, 'f3': 0}
LST = {'dil': LC, 'f1': T1, 'f2': T2, 'f3': T3}


def _wlen(stage, j):
    """columns stage tile j computes: [STEP*j, STEP*j + wlen)"""
    t0 = STEP * j
    if t0 >= LST[stage]:
        return 0
    return min(t0 + STEP + EXT[stage], LST[stage]) - t0


def build_bass():
    import bass_rust
    import concourse.bass as bass
    import concourse.mybir as mybir
    from concourse.tile import TileContext

    dt = mybir.dt
    AF = mybir.ActivationFunctionType
    ALU = mybir.AluOpType

    nc = bass.Bass()

    p = {}
    p['x'] = nc.declare_dram_parameter('x', [BS * 9, LX], dt.bfloat16, False)
    p['y'] = nc.declare_dram_parameter('y', [BS * 3, LX], dt.bfloat16, False)
    for nm, sh in [('hT1', [BS * 9, BS * 6]), ('hT2', [BS * 6, BS * 4]),
                   ('hT3', [BS * 4, BS * 2]),
                   ('hR1', [BS * 3, BS * 2]), ('hR2', [BS * 2, BS * 2]),
                   ('dilA', [52, 100]), ('dilB', [52, 100]),
                   ('F1', [6, 100, 48]), ('F2', [6, 112, 40]),
                   ('F3', [6, 40, 4])]:
        p[nm] = nc.declare_dram_parameter(nm, sh, dt.bfloat16, False)
    for nm, sh in [('hbT1', [BS * 6, 1]), ('hbT2', [BS * 4, 1]),
                   ('hbT3', [BS * 2, 1]),
                   ('hbR1', [BS * 2, 1]), ('hbR2', [BS * 2, 1]),
                   ('bMA', [100, 1]), ('bMB', [100, 1]),
                   ('bF1', [112, 1]), ('bF2', [104, 1]), ('thr', [36, 1])]:
        p[nm] = nc.declare_dram_parameter(nm, sh, dt.float32, False)
    out_d = nc.declare_dram_parameter('out', [BS * 4, T3], dt.bfloat16, True)
    # DRAM staging for o2t: window reads from HBM avoid the SBUF
    # per-partition read bottleneck (26x overlapping reads of one row)
    o2d = nc.dram_tensor('o2d', [BS * 2, L], dt.bfloat16)

    with TileContext(nc) as tc:
        with tc.tile_pool(name="wpool", bufs=1) as wp, \
             tc.tile_pool(name="head", bufs=1) as hp, \
             tc.tile_pool(name="ring", bufs=1) as rp, \
             tc.tile_pool(name="psum", bufs=1, space="PSUM") as pp:

            # ---------------- weights to SBUF ----------------
            # head weights + inputs on the sync queue (head critical path);
            # stage weights on gpsimd (hidden under the head)
            xt = hp.tile([BS * 9, LX], dt.bfloat16, name="xt")
            yt = hp.tile([BS * 3, LX], dt.bfloat16, name="yt")
            nc.sync.dma_start(out=xt[...], in_=p['x'][...])
            W = {}
            for nm in ('hT1', 'hbT1', 'hR1', 'hbR1', 'hT2', 'hbT2',
                       'hR2', 'hbR2', 'hT3', 'hbT3'):
                t = wp.tile(list(p[nm].shape), p[nm].dtype, name=f"w_{nm}")
                nc.sync.dma_start(out=t[...], in_=p[nm][...])
                W[nm] = t
            for nm in ('dilA', 'bMA', 'bMB'):
                t = wp.tile(list(p[nm].shape), p[nm].dtype, name=f"w_{nm}")
                nc.sync.dma_start(out=t[...], in_=p[nm][...])
                W[nm] = t
            for nm in ('bF1', 'bF2', 'thr'):
                t = wp.tile(list(p[nm].shape), p[nm].dtype, name=f"w_{nm}")
                nc.gpsimd.dma_start(out=t[...], in_=p[nm][...])
                W[nm] = t
            # dilB duplicated at partitions 64..115 for row tiling
            w_dilB = wp.tile([116, 100], dt.bfloat16, name="w_dilB")
            nc.sync.dma_start(out=w_dilB[64:116], in_=p['dilB'][...])
            W['dilB'] = w_dilB
            for nm in ('F1', 'F2', 'F3'):
                n_sl, kk, mm_ = p[nm].shape
                W[nm] = []
                for i_sl in range(n_sl):
                    t = wp.tile([kk, mm_], p[nm].dtype, name=f"w_{nm}{i_sl}")
                    nc.gpsimd.dma_start(out=t[...], in_=p[nm][i_sl])
                    W[nm].append(t)
            # F3 weights duplicated at partitions 64..103 for row tiling
            W['F3d'] = []
            for i_sl in range(6):
                t = wp.tile([104, 4], dt.bfloat16, name=f"w_F3d{i_sl}")
                nc.gpsimd.dma_start(out=t[64:104], in_=p['F3'][i_sl])
                W['F3d'].append(t)

            # ---------------- static PSUM tiles (8 banks) ----------------
            P_DA = pp.tile([128, WIN], dt.float32, name="P_DA")    # 2 banks
            P_DB = pp.tile([128, NT], dt.float32, name="P_DB")     # 1 bank
            P_F1 = pp.tile([128, WIN], dt.float32, name="P_F1")    # 2 banks
            P_F2 = pp.tile([128, WIN], dt.float32, name="P_F2")    # 2 banks
            P_F3 = pp.tile([128, NT], dt.float32, name="P_F3")     # 1 bank

            # one-time zero of the F1 gap rows (never matmul-written)
            nc.vector.memset(P_F1[32:64, :], 0.0)

            # ---------------- head ----------------
            nc.sync.dma_start(out=yt[...], in_=p['y'][...])
            o2t = hp.tile([BS * 2, L], dt.bfloat16, name="o2t")
            head_ps = [P_DA, P_F1, P_F2]

            def head_layer(w_nm, b_nm, rows_in, rows_out, src, dst, act,
                           dst_off=0):
                for j in range(LX // WIN):
                    pt = head_ps[j % 3]
                    for h in range(2):
                        sl = slice(j * WIN + h * NT, j * WIN + (h + 1) * NT)
                        nc.tensor.matmul(pt[0:rows_out, h * NT:(h + 1) * NT],
                                         W[w_nm][...], src[:rows_in, sl],
                                         start=True, stop=True)
                    sl2 = slice(dst_off + j * WIN, dst_off + (j + 1) * WIN)
                    ps = pt[0:rows_out, 0:WIN]
                    if act == 'relu':
                        nc.scalar.activation(dst[:rows_out, sl2], ps,
                                             AF.Relu, bias=W[b_nm][...])
                    else:
                        nc.vector.tensor_scalar(dst[:rows_out, sl2], ps,
                                                W[b_nm][...], None, ALU.add)

            a1h = hp.tile([BS * 6, LX], dt.bfloat16, name="a1h")
            a2h = hp.tile([BS * 6, LX], dt.bfloat16, name="a2h")
            b1h = hp.tile([BS * 6, LX], dt.bfloat16, name="b1h")
            head_layer('hT1', 'hbT1', BS * 9, BS * 6, xt, a1h, 'relu')
            head_layer('hT2', 'hbT2', BS * 6, BS * 4, a1h, a2h, 'relu')
            head_layer('hT3', 'hbT3', BS * 4, BS * 2, a2h, o2t, 'add')
            # stage the T half to DRAM while the R path still computes
            nc.sync.dma_start(out=o2d[:, 0:LX], in_=o2t[:, 0:LX])
            head_layer('hR1', 'hbR1', BS * 3, BS * 2, yt, b1h, 'relu')
            head_layer('hR2', 'hbR2', BS * 2, BS * 2, b1h, o2t, 'relu',
                       dst_off=LX)
            nc.sync.dma_start(out=o2d[:, LX:L], in_=o2t[:, LX:L])

            # ---------------- ring buffers ----------------
            S_bufs = [rp.tile([116, LC], dt.bfloat16, name=f"S{i}")
                      for i in range(4)]
            # per-sample rings: slot = 2*(s%2) + (t%2), t = 9*pair + j
            Oa = [rp.tile([100, WIN], dt.bfloat16, name=f"Oa{i}")
                  for i in range(4)]
            Ob = [rp.tile([100, WIN], dt.bfloat16, name=f"Ob{i}")
                  for i in range(4)]
            a1r = [rp.tile([112, WIN], dt.bfloat16, name=f"a1r{i}")
                   for i in range(4)]
            # per-pair rings: slot = t%2
            a2r = [rp.tile([104, WIN], dt.bfloat16, name=f"a2r{i}")
                   for i in range(2)]
            of3 = [rp.tile([36, NT], dt.bfloat16, name=f"of3{i}")
                   for i in range(2)]
            nbt = [rp.tile([36, NT], dt.float32, name=f"nbt{i}")
                   for i in range(2)]

            def prefetch_S(s, dup_dram=True):
                """window DMAs (from DRAM staging) building S for sample s,
                primary rows c*26+w plus the row-64 duplicate."""
                St = S_bufs[s % 4]
                qs = [nc.sync, nc.gpsimd]
                for c in range(2):
                    for dd in range(2):
                        win = o2d[s * 2 + c:s * 2 + c + 1, 0:LC].copy()
                        win.ap = bass_rust.VecI64Pair(
                            [[L, 1], [1, 26], [1, LC]])
                        r0 = 64 * dd + c * 26
                        qs[(c + dd) % 2].dma_start(
                            out=St[r0:r0 + 26, :], in_=win)

            def halves(ln):
                out = [(0, min(NT, ln))]
                if ln > NT:
                    out.append((NT, ln))
                return out

            # ---------------- stage emitters ----------------
            def dil_mm(s, j, half):
                ln = _wlen('dil', j)
                hs = halves(ln)
                if half >= len(hs):
                    return
                lo, hi = hs[half]
                St = S_bufs[s % 4]
                t0 = STEP * j
                nc.tensor.matmul(P_DA[0:100, lo:hi], W['dilA'][...],
                                 St[0:52, t0 + lo:t0 + hi],
                                 start=True, stop=True, tile_position=(0, 0),
                                 skip_group_check=True)
                nc.tensor.matmul(P_DB[0:100, 0:hi - lo], W['dilB'][64:116],
                                 St[64:116, t0 + lo:t0 + hi],
                                 start=True, stop=True, tile_position=(64, 0),
                                 skip_group_check=True)

            def dil_post(s, j, slot, half):
                ln = _wlen('dil', j)
                hs = halves(ln)
                if half >= len(hs):
                    return
                lo, hi = hs[half]
                nc.vector.tensor_scalar(Ob[slot][:, lo:hi],
                                        P_DB[0:100, 0:hi - lo],
                                        W['bMB'][...], 0.0, ALU.add, ALU.max)
                if half == len(hs) - 1:
                    nc.scalar.activation(Oa[slot][:, 0:ln], P_DA[0:100, 0:ln],
                                         AF.Relu, bias=W['bMA'][...])

            def f1_mm(s, j, slot, half):
                ln = _wlen('f1', j)
                hs = halves(ln)
                if half >= len(hs):
                    return
                lo, hi = hs[half]
                for dw in range(6):
                    nc.tensor.matmul(P_F1[0:48, lo:hi], W['F1'][dw][...],
                                     Oa[slot][:, lo + dw:hi + dw],
                                     start=(dw == 0), stop=(dw == 5),
                                     skip_group_check=True)
                    nc.tensor.matmul(P_F1[64:112, lo:hi], W['F1'][dw][...],
                                     Ob[slot][:, lo + dw:hi + dw],
                                     start=(dw == 0), stop=(dw == 5),
                                     skip_group_check=True)

            def f1_post(s, j, slot):
                ln = _wlen('f1', j)
                nc.scalar.activation(a1r[slot][:, 0:ln], P_F1[0:112, 0:ln],
                                     AF.Sigmoid, bias=W['bF1'][...])

            def f2_mm(pair, j, t, half):
                ln = _wlen('f2', j)
                hs = halves(ln)
                if half >= len(hs):
                    return
                lo, hi = hs[half]
                sl0 = 2 * 0 + (t % 2)   # sample 2*pair
                sl1 = 2 * 1 + (t % 2)   # sample 2*pair+1
                for dw in range(6):
                    nc.tensor.matmul(P_F2[0:40, lo:hi], W['F2'][dw][...],
                                     a1r[sl0][:, lo + dw:hi + dw],
                                     start=(dw == 0), stop=(dw == 5),
                                     skip_group_check=True)
                    nc.tensor.matmul(P_F2[64:104, lo:hi], W['F2'][dw][...],
                                     a1r[sl1][:, lo + dw:hi + dw],
                                     start=(dw == 0), stop=(dw == 5),
                                     skip_group_check=True)

            def f2_post(pair, j, t):
                ln = _wlen('f2', j)
                slot = t % 2
                nc.scalar.activation(a2r[slot][:, 0:ln], P_F2[0:104, 0:ln],
                                     AF.Sigmoid, bias=W['bF2'][...])

            def f3_half(pair, j, t, half):
                ln = _wlen('f3', j)
                hs = halves(ln)
                if half >= len(hs):
                    return
                lo, hi = hs[half]
                slot = t % 2
                t0 = STEP * j
                # 4-way packed: pA chains (dw 0..2) at partitions 0..3 (s0)
                # and 32..35 (s1); nB chains (negated dw 3..5) at 64..67 /
                # 96..99; fused (pA - thr) > nB threshold
                for dwl in range(3):
                    for si in range(2):
                        dwa, dwb = dwl, 3 + dwl
                        rhs = a2r[slot][64 * si:64 * si + 40, :]
                        wA = W['F3'][dwa][...] if si == 0 \
                            else W['F3d'][dwa][64:104]
                        wB = W['F3'][dwb][...] if si == 0 \
                            else W['F3d'][dwb][64:104]
                        nc.tensor.matmul(
                            P_F3[32 * si:32 * si + 4, 0:hi - lo], wA,
                            rhs[:, lo + dwa:hi + dwa],
                            start=(dwl == 0), stop=(dwl == 2),
                            tile_position=(64 * si, 32 * si),
                            skip_group_check=True)
                        nc.tensor.matmul(
                            P_F3[64 + 32 * si:64 + 32 * si + 4, 0:hi - lo], wB,
                            rhs[:, lo + dwb:hi + dwb],
                            start=(dwl == 0), stop=(dwl == 2),
                            tile_position=(64 * si, 64 + 32 * si),
                            skip_group_check=True)
                nc.scalar.activation(nbt[slot][:, 0:hi - lo],
                                     P_F3[64:100, 0:hi - lo], AF.Copy)
                nc.vector.scalar_tensor_tensor(
                    of3[slot][:, 0:hi - lo], P_F3[0:36, 0:hi - lo],
                    W['thr'][...], nbt[slot][:, 0:hi - lo],
                    ALU.subtract, ALU.is_gt)
                for si in range(2):
                    s = pair * 2 + si
                    nc.gpsimd.dma_start(
                        out=out_d[s * 4:(s + 1) * 4, t0 + lo:t0 + hi],
                        in_=of3[slot][si * 32:si * 32 + 4, 0:hi - lo])

            # ---------------- skewed global pipeline ----------------
            NTOT = NPAIR * NSUP           # 36 global supertiles
            for s in (0, 1, 2, 3):
                prefetch_S(s, dup_dram=(s < 2))

            def stage_pt(g, skew):
                tt = g - skew
                if 0 <= tt < NTOT:
                    return tt, tt // NSUP, tt % NSUP
                return None, None, None

            for g in range(NTOT + 3):
                tD, pD, jD = stage_pt(g, 0)
                t1_, p1, j1 = stage_pt(g, 1)
                t2_, p2, j2 = stage_pt(g, 2)
                t3_, p3, j3 = stage_pt(g, 3)

                # S prefetch for the next pair, early in the pair
                if tD is not None and jD == 2 and pD < NPAIR - 1:
                    prefetch_S(2 * (pD + 1))
                    prefetch_S(2 * (pD + 1) + 1)

                sD0 = 2 * pD if tD is not None else None
                sD1 = sD0 + 1 if tD is not None else None
                slD0 = (tD % 2) if tD is not None else None
                slD1 = 2 + (tD % 2) if tD is not None else None
                sF0 = 2 * p1 if t1_ is not None else None
                sF1 = sF0 + 1 if t1_ is not None else None
                slF0 = (t1_ % 2) if t1_ is not None else None
                slF1 = 2 + (t1_ % 2) if t1_ is not None else None

                if tD is not None:
                    dil_mm(sD0, jD, 0)
                if t1_ is not None:
                    f1_mm(sF0, j1, slF0, 0)
                if tD is not None:
                    dil_post(sD0, jD, slD0, 0)
                    dil_mm(sD0, jD, 1)
                if t2_ is not None:
                    f2_mm(p2, j2, t2_, 0)
                if t1_ is not None:
                    f1_mm(sF0, j1, slF0, 1)
                    f1_post(sF0, j1, slF0)
                if tD is not None:
                    dil_post(sD0, jD, slD0, 1)
                if t3_ is not None:
                    f3_half(p3, j3, t3_, 0)
                if tD is not None:
                    dil_mm(sD1, jD, 0)
                if t2_ is not None:
                    f2_mm(p2, j2, t2_, 1)
                if t1_ is not None:
                    f1_mm(sF1, j1, slF1, 0)
                if tD is not None:
                    dil_post(sD1, jD, slD1, 0)
                    dil_mm(sD1, jD, 1)
                if t1_ is not None:
                    f1_mm(sF1, j1, slF1, 1)
                    f1_post(sF1, j1, slF1)
                if tD is not None:
                    dil_post(sD1, jD, slD1, 1)
                if t2_ is not None:
                    f2_post(p2, j2, t2_)
                if t3_ is not None:
                    f3_half(p3, j3, t3_, 1)

    return _patch_serialization(nc)


def kernel(**inputs):
    inputs = {k: np.asarray(v) for k, v in inputs.items()}
    in_maps = _shard_inputs(inputs)
    nc = build_bass()
    from concourse.bass_utils import run_bass_kernel_spmd
    res = run_bass_kernel_spmd(nc, in_maps, core_ids=list(range(NCORES)))
    outs = [res.results[i]['out'].reshape(BS, 4, T3) for i in range(NCORES)]
    full = np.concatenate(outs, axis=0)[:, None]  # [64, 1, 4, T3]
    return full.astype(np.float32)


# revision 43
# speedup vs baseline: 1.3154x; 1.0204x over previous
"""Trainium2 Bass kernel for nn_Allocator2 (dense_cnn), 8 NeuronCores.

Pure data parallelism: batch 64 -> 8 samples per core, weights replicated.

v2 redesign around PE array packing + a skewed windowed pipeline:

  head : 1x1 convs packed across 8 samples with block-diagonal weights,
         PSUM borrowed from the stage tiles.
  dil  : output in hh-major layout, two chunks with a duplicated row band:
           A = (hh 0..3) x 25 branches = 100 rows (r = 25*hh + i)
           B = (hh 3..6) x 25 branches = 100 rows
         row-packed: A at array rows 0..51, B at rows 64..115 (S and the
         B weights are duplicated at partitions 64+), so the two matmuls
         run concurrently in disjoint row groups.
  F1   : two chains, h in {0,1,2} on chunk A and h in {3,4,5} on chunk B,
         each K=100, M=48 (m = 16*h_loc + o).  The chains share the same
         lhsT (translation invariance in h) and run concurrently in col
         groups (out partitions 0..47 / 64..111), interleaved
         accumulation chains in one PSUM bank pair.
  F2   : K=112 over the gapped a1 layout (zero weights in rows 48..63),
         M=40 (m = 5*o + h), 2-sample col packing (out partitions 0..39
         and 64..103), one shared a2 pair tile + single ACT.
  F3   : baked K=120 (3 phases) x 2 taps, 2-sample col packing, single
         vector threshold per tile (round(sigmoid) == z > -bF3).

  Stage buffers are ~1KB ring slots stepped by STEP=1008 with cascaded
  extra columns (dil +15, F1 +10, F2 +5, bake +3) so each consumer tile j
  reads only producer slot j.  Global skewed pipeline over 36 (pair,
  supertile) ticks: tick g runs dil(g), F1(g-1), F2+bake(g-2), F3(g-3),
  so the PE never drains between stages, samples, or pairs.
"""

import numpy as np
import ml_dtypes

BF16 = ml_dtypes.bfloat16

B = 64            # global batch
NCORES = 8
BS = B // NCORES  # 8 samples per core
ND = 25
L = 8192          # concat length (4096 + 4096)
LX = 4096
LC = L - ND       # 8167 dilated output length
T1 = LC - 5       # 8162 F1 output length
T2 = T1 - 5       # 8157
T3 = T2 - 5       # 8152
NT = 512          # matmul free-dim tile
STEP = 1008       # supertile step
WIN = 1024        # ring slot width (2 PSUM banks of fp32)
NSUP = -(-LC // STEP)   # 9 supertiles per sample
NPAIR = BS // 2


def _bd(blocks):
    """block-diagonal stack of 2D arrays"""
    rs = sum(b.shape[0] for b in blocks)
    cs = sum(b.shape[1] for b in blocks)
    out = np.zeros((rs, cs), np.float32)
    r = c = 0
    for b in blocks:
        out[r:r + b.shape[0], c:c + b.shape[1]] = b
        r += b.shape[0]
        c += b.shape[1]
    return out


def build_weights(inp):
    """Host-side weight prep. Returns dict of np arrays (bf16 weights,
    fp32 biases) shared by all cores."""
    w = {}
    f32 = np.float32

    # ---- head: block-diagonal over BS samples, lhsT layout [K, M] ----
    def head_lhsT(wmat):  # wmat [Co, Ci] -> lhsT [Ci, Co] per sample
        return _bd([wmat.T.astype(f32)] * BS)

    w['hT1'] = head_lhsT(inp['wT1'])   # [72, 48]
    w['hT2'] = head_lhsT(inp['wT2'])   # [48, 32]
    w['hT3'] = head_lhsT(inp['wT3'])   # [32, 16]
    w['hR1'] = head_lhsT(inp['wR1'])   # [24, 16]
    w['hR2'] = head_lhsT(inp['wR2'])   # [16, 16]
    for nm in ('bT1', 'bT2', 'bT3', 'bR1', 'bR2'):
        w['hb' + nm[1:]] = np.tile(inp[nm].astype(f32), BS)[:, None]

    # ---- dil: original [52, 175], cols remapped to hh-major chunks ----
    dil = np.zeros((52, 175), f32)
    wM = inp['wM'].astype(f32)  # [25, 7, 2, 2]
    for i in range(ND):
        for o in range(7):
            m = i * 7 + o
            for c in range(2):
                dil[c * 26 + 0, m] = wM[i, o, c, 0]          # shift 0 tap
                dil[c * 26 + (i + 1), m] = wM[i, o, c, 1]    # shift i+1 tap
    dilA = np.zeros((52, 100), f32)   # cols 25*o + i, o in 0..3
    dilB = np.zeros((52, 100), f32)   # cols 25*(o-3) + i, o in 3..6
    bMA = np.zeros((100,), f32)
    bMB = np.zeros((100,), f32)
    for i in range(ND):
        for o in range(7):
            if o < 4:
                dilA[:, 25 * o + i] = dil[:, i * 7 + o]
                bMA[25 * o + i] = inp['bM'][i, o]
            if o >= 3:
                dilB[:, 25 * (o - 3) + i] = dil[:, i * 7 + o]
                bMB[25 * (o - 3) + i] = inp['bM'][i, o]
    w['dilA'] = dilA
    w['dilB'] = dilB
    w['bMA'] = bMA[:, None]
    w['bMB'] = bMB[:, None]

    # ---- F1: 6 lhsT [100, 48], shared by both chains ----
    wF1 = inp['wF1'].astype(f32)  # [16, 25, 2, 6]
    f1 = np.zeros((6, 100, 48), f32)
    for dw in range(6):
        for hl in range(3):
            for hhl in range(4):
                dh = hhl - hl
                if 0 <= dh <= 1:
                    for o in range(16):
                        for i in range(ND):
                            f1[dw, 25 * hhl + i, 16 * hl + o] = wF1[o, i, dh, dw]
    w['F1'] = f1
    # bias in a1 layout [112]: h<3 -> 16h+o ; h>=3 -> 64+16(h-3)+o
    bF1 = np.zeros((112,), f32)
    for h in range(6):
        r0 = 16 * h if h < 3 else 64 + 16 * (h - 3)
        for o in range(16):
            bF1[r0 + o] = inp['bF1'][o]
    w['bF1'] = bF1[:, None]

    # ---- F2: 6 lhsT [112, 40] over gapped a1 rows, m = 5*o + h ----
    wF2 = inp['wF2'].astype(f32)  # [8, 16, 2, 6]
    f2 = np.zeros((6, 112, 40), f32)
    for dw in range(6):
        for hh in range(6):
            r0 = 16 * hh if hh < 3 else 64 + 16 * (hh - 3)
            for ci in range(16):
                for o in range(8):
                    for h in range(5):
                        dh = hh - h
                        if 0 <= dh <= 1:
                            f2[dw, r0 + ci, o * 5 + h] = wF2[o, ci, dh, dw]
    w['F2'] = f2
    bF2 = np.zeros((104,), f32)
    for o in range(8):
        for h in range(5):
            bF2[o * 5 + h] = inp['bF2'][o]
            bF2[64 + o * 5 + h] = inp['bF2'][o]
    w['bF2'] = bF2[:, None]

    # ---- F3 direct: lhsT[dw] [40, 4]; row m = ci*5 + hh, col h ----
    # dw 3..5 negated: the fused threshold computes (pA - thr) > nB with
    # nB = -sum_{dw>=3} W.T a2, i.e. pA + pB > thr.
    wF3 = inp['wF3'].astype(f32)  # [1, 8, 2, 6]
    f3 = np.zeros((6, 40, 4), f32)
    for dw in range(6):
        sgn = 1.0 if dw < 3 else -1.0
        for ci in range(8):
            for hh in range(5):
                for h in range(4):
                    dh = hh - h
                    if 0 <= dh <= 1:
                        f3[dw, ci * 5 + hh, h] = sgn * wF3[0, ci, dh, dw]
    w['F3'] = f3
    thr = np.zeros((36,), f32)
    thr[0:4] = -inp['bF3'][0]
    thr[32:36] = -inp['bF3'][0]
    w['thr'] = thr[:, None]

    # bf16-ify matmul weights
    for k in ('hT1', 'hT2', 'hT3', 'hR1', 'hR2', 'dilA', 'dilB',
              'F1', 'F2', 'F3'):
        w[k] = w[k].astype(BF16)
    return w


def emulate_core(w, x_core, y_core):
    """Numpy emulation of exactly what the Bass kernel computes for one
    core. x_core [72, 4096] bf16, y_core [24, 4096] bf16. Returns
    [BS, 4, T3] f32 in {0,1}."""
    f32 = np.float32

    def mm(lhsT, rhs):  # bf16 operands, f32 accumulate
        return lhsT.astype(f32).T @ rhs.astype(f32)

    relu = lambda a: np.maximum(a, 0)
    sig = lambda a: 1.0 / (1.0 + np.exp(-a))

    a = relu(mm(w['hT1'], x_core) + w['hbT1']).astype(BF16)
    a = relu(mm(w['hT2'], a) + w['hbT2']).astype(BF16)
    t3 = (mm(w['hT3'], a) + w['hbT3']).astype(BF16)          # [16, 4096]
    b_ = relu(mm(w['hR1'], y_core) + w['hbR1']).astype(BF16)
    b_ = relu(mm(w['hR2'], b_) + w['hbR2']).astype(BF16)     # [16, 4096]
    out2 = np.concatenate([t3, b_], axis=1)                  # [16, 8192] bf16

    res = np.zeros((BS, 4, T3), f32)
    for s in range(BS):
        o2 = out2[s * 2:s * 2 + 2]                           # [2, 8192]
        S = np.zeros((52, LC), BF16)
        for c in range(2):
            for sh in range(26):
                S[c * 26 + sh] = o2[c, sh:sh + LC]
        OA = relu(mm(w['dilA'], S) + w['bMA']).astype(BF16)   # [100, LC]
        OB = relu(mm(w['dilB'], S) + w['bMB']).astype(BF16)   # [100, LC]
        z1 = np.zeros((112, T1), f32)
        for dw in range(6):
            z1[0:48] += mm(w['F1'][dw], OA[:, dw:dw + T1])
            z1[64:112] += mm(w['F1'][dw], OB[:, dw:dw + T1])
        a1 = sig(z1 + w['bF1']).astype(BF16)                 # [112, T1]
        a1[48:64] = 0.0                                      # gap rows
        z2 = np.zeros((40, T2), f32)
        for dw in range(6):
            z2 += mm(w['F2'][dw], a1[:, dw:dw + T2])
        a2 = sig(z2 + w['bF2'][0:40]).astype(BF16)           # [40, T2]
        pA = np.zeros((4, T3), f32)
        nB = np.zeros((4, T3), f32)
        for dw in range(3):
            pA += mm(w['F3'][dw], a2[:, dw:dw + T3])
        for dw in range(3, 6):
            nB += mm(w['F3'][dw], a2[:, dw:dw + T3])
        res[s] = ((pA - w['thr'][0:4]) > nB).astype(f32)     # [4, T3]
    return res


def _shard_inputs(inputs):
    """Build per-core in_maps (host-side prep + shard)."""
    w = build_weights(inputs)
    in_maps = []
    for c in range(NCORES):
        m = dict(w)
        xs = inputs['x'][c * BS:(c + 1) * BS]  # [8, 9, 4096]
        ys = inputs['y'][c * BS:(c + 1) * BS]
        m['x'] = np.ascontiguousarray(xs.reshape(BS * 9, LX)).astype(BF16)
        m['y'] = np.ascontiguousarray(ys.reshape(BS * 3, LX)).astype(BF16)
        in_maps.append(m)
    return in_maps


# ---------------------------------------------------------------------------
# Bass program
# ---------------------------------------------------------------------------

def _split_excess_waits(bir, maxw=1):
    """The walrus build in this container refuses instructions carrying
    more than ~1 semaphore wait ("Too many sync wait commands").  Tile
    attaches multi-waits freely.  Splitting is semantics-preserving: move
    excess waits onto injected NoOps on the same engine immediately
    before the instruction (engines execute their instruction stream in
    order, so wait-all is preserved)."""
    for fn in bir['functions']:
        for bb in fn['blocks']:
            out = []
            for inst in bb['instructions']:
                si = inst.get('sync_info')
                waits = (si or {}).get('on_wait') or []
                if len(waits) > maxw:
                    extra, keep = waits[:-maxw], waits[-maxw:]
                    for i in range(0, len(extra), maxw):
                        out.append({
                            "debug": inst.get("debug", 0),
                            "engine": inst["engine"], "ins": [],
                            "name": f"{inst['name']}-wsplit{i}",
                            "opcode": "NoOp", "outs": [],
                            "sync_info": {"on_update": [],
                                          "on_wait": extra[i:i + maxw]}})
                    si['on_wait'] = keep
                out.append(inst)
            bb['instructions'] = out
    return bir


def _patch_serialization(nc):
    import orjson
    bir = _split_excess_waits(nc.to_json())
    patched = orjson.dumps(bir)
    nc.to_json_bytes = lambda: patched
    return nc


# per-stage output lengths and window extensions
EXT = {'dil': 15, 'f1': 10, 'f2': 5, 'f3': 0}
LST = {'dil': LC, 'f1': T1, 'f2': T2, 'f3': T3}


def _wlen(stage, j):
    """columns stage tile j computes: [STEP*j, STEP*j + wlen)"""
    t0 = STEP * j
    if t0 >= LST[stage]:
        return 0
    return min(t0 + STEP + EXT[stage], LST[stage]) - t0


def build_bass():
    import bass_rust
    import concourse.bass as bass
    import concourse.mybir as mybir
    from concourse.tile import TileContext

    dt = mybir.dt
    AF = mybir.ActivationFunctionType
    ALU = mybir.AluOpType

    nc = bass.Bass()

    p = {}
    p['x'] = nc.declare_dram_parameter('x', [BS * 9, LX], dt.bfloat16, False)
    p['y'] = nc.declare_dram_parameter('y', [BS * 3, LX], dt.bfloat16, False)
    for nm, sh in [('hT1', [BS * 9, BS * 6]), ('hT2', [BS * 6, BS * 4]),
                   ('hT3', [BS * 4, BS * 2]),
                   ('hR1', [BS * 3, BS * 2]), ('hR2', [BS * 2, BS * 2]),
                   ('dilA', [52, 100]), ('dilB', [52, 100]),
                   ('F1', [6, 100, 48]), ('F2', [6, 112, 40]),
                   ('F3', [6, 40, 4])]:
        p[nm] = nc.declare_dram_parameter(nm, sh, dt.bfloat16, False)
    for nm, sh in [('hbT1', [BS * 6, 1]), ('hbT2', [BS * 4, 1]),
                   ('hbT3', [BS * 2, 1]),
                   ('hbR1', [BS * 2, 1]), ('hbR2', [BS * 2, 1]),
                   ('bMA', [100, 1]), ('bMB', [100, 1]),
                   ('bF1', [112, 1]), ('bF2', [104, 1]), ('thr', [36, 1])]:
        p[nm] = nc.declare_dram_parameter(nm, sh, dt.float32, False)
    out_d = nc.declare_dram_parameter('out', [BS * 4, T3], dt.bfloat16, True)
    # DRAM staging for o2t: window reads from HBM avoid the SBUF
    # per-partition read bottleneck (26x overlapping reads of one row)
    o2d = nc.dram_tensor('o2d', [BS * 2, L], dt.bfloat16)

    with TileContext(nc) as tc:
        with tc.tile_pool(name="wpool", bufs=1) as wp, \
             tc.tile_pool(name="head", bufs=1) as hp, \
             tc.tile_pool(name="ring", bufs=1) as rp, \
             tc.tile_pool(name="psum", bufs=1, space="PSUM") as pp:

            # ---------------- weights to SBUF ----------------
            # head weights + inputs on the sync queue (head critical path);
            # stage weights on gpsimd (hidden under the head)
            xt = hp.tile([BS * 9, LX], dt.bfloat16, name="xt")
            yt = hp.tile([BS * 3, LX], dt.bfloat16, name="yt")
            nc.sync.dma_start(out=xt[...], in_=p['x'][...])
            W = {}
            for nm in ('hT1', 'hbT1', 'hR1', 'hbR1', 'hT2', 'hbT2',
                       'hR2', 'hbR2', 'hT3', 'hbT3'):
                t = wp.tile(list(p[nm].shape), p[nm].dtype, name=f"w_{nm}")
                nc.sync.dma_start(out=t[...], in_=p[nm][...])
                W[nm] = t
            for nm in ('dilA', 'bMA', 'bMB'):
                t = wp.tile(list(p[nm].shape), p[nm].dtype, name=f"w_{nm}")
                nc.sync.dma_start(out=t[...], in_=p[nm][...])
                W[nm] = t
            for nm in ('bF1', 'bF2', 'thr'):
                t = wp.tile(list(p[nm].shape), p[nm].dtype, name=f"w_{nm}")
                nc.gpsimd.dma_start(out=t[...], in_=p[nm][...])
                W[nm] = t
            # dilB duplicated at partitions 64..115 for row tiling
            w_dilB = wp.tile([116, 100], dt.bfloat16, name="w_dilB")
            nc.sync.dma_start(out=w_dilB[64:116], in_=p['dilB'][...])
            W['dilB'] = w_dilB
            for nm in ('F1', 'F2', 'F3'):
                n_sl, kk, mm_ = p[nm].shape
                W[nm] = []
                for i_sl in range(n_sl):
                    t = wp.tile([kk, mm_], p[nm].dtype, name=f"w_{nm}{i_sl}")
                    nc.gpsimd.dma_start(out=t[...], in_=p[nm][i_sl])
                    W[nm].append(t)
            # F3 weights duplicated at partitions 64..103 for row tiling
            W['F3d'] = []
            for i_sl in range(6):
                t = wp.tile([104, 4], dt.bfloat16, name=f"w_F3d{i_sl}")
                nc.gpsimd.dma_start(out=t[64:104], in_=p['F3'][i_sl])
                W['F3d'].append(t)

            # ---------------- static PSUM tiles (8 banks) ----------------
            P_DA = pp.tile([128, WIN], dt.float32, name="P_DA")    # 2 banks
            P_DB = pp.tile([128, NT], dt.float32, name="P_DB")     # 1 bank
            P_F1 = pp.tile([128, WIN], dt.float32, name="P_F1")    # 2 banks
            P_F2 = pp.tile([128, WIN], dt.float32, name="P_F2")    # 2 banks
            P_F3 = pp.tile([128, NT], dt.float32, name="P_F3")     # 1 bank

            # one-time zero of the F1 gap rows (never matmul-written)
            nc.vector.memset(P_F1[32:64, :], 0.0)

            # ---------------- head ----------------
            nc.sync.dma_start(out=yt[...], in_=p['y'][...])
            o2t = hp.tile([BS * 2, L], dt.bfloat16, name="o2t")
            head_ps = [P_DA, P_F1, P_F2]

            def head_layer(w_nm, b_nm, rows_in, rows_out, src, dst, act,
                           dst_off=0):
                for j in range(LX // WIN):
                    pt = head_ps[j % 3]
                    for h in range(2):
                        sl = slice(j * WIN + h * NT, j * WIN + (h + 1) * NT)
                        nc.tensor.matmul(pt[0:rows_out, h * NT:(h + 1) * NT],
                                         W[w_nm][...], src[:rows_in, sl],
                                         start=True, stop=True)
                    sl2 = slice(dst_off + j * WIN, dst_off + (j + 1) * WIN)
                    ps = pt[0:rows_out, 0:WIN]
                    if act == 'relu':
                        nc.scalar.activation(dst[:rows_out, sl2], ps,
                                             AF.Relu, bias=W[b_nm][...])
                    else:
                        nc.vector.tensor_scalar(dst[:rows_out, sl2], ps,
                                                W[b_nm][...], None, ALU.add)

            a1h = hp.tile([BS * 6, LX], dt.bfloat16, name="a1h")
            a2h = hp.tile([BS * 6, LX], dt.bfloat16, name="a2h")
            b1h = hp.tile([BS * 6, LX], dt.bfloat16, name="b1h")
            head_layer('hT1', 'hbT1', BS * 9, BS * 6, xt, a1h, 'relu')
            head_layer('hT2', 'hbT2', BS * 6, BS * 4, a1h, a2h, 'relu')
            head_layer('hT3', 'hbT3', BS * 4, BS * 2, a2h, o2t, 'add')
            # stage the T half to DRAM while the R path still computes
            nc.sync.dma_start(out=o2d[:, 0:LX], in_=o2t[:, 0:LX])
            head_layer('hR1', 'hbR1', BS * 3, BS * 2, yt, b1h, 'relu')
            head_layer('hR2', 'hbR2', BS * 2, BS * 2, b1h, o2t, 'relu',
                       dst_off=LX)
            nc.sync.dma_start(out=o2d[:, LX:L], in_=o2t[:, LX:L])

            # ---------------- ring buffers ----------------
            S_bufs = [rp.tile([116, LC], dt.bfloat16, name=f"S{i}")
                      for i in range(4)]
            # per-sample rings: slot = 2*(s%2) + (t%2), t = 9*pair + j
            Oa = [rp.tile([100, WIN], dt.bfloat16, name=f"Oa{i}")
                  for i in range(4)]
            Ob = [rp.tile([100, WIN], dt.bfloat16, name=f"Ob{i}")
                  for i in range(4)]
            a1r = [rp.tile([112, WIN], dt.bfloat16, name=f"a1r{i}")
                   for i in range(4)]
            # per-pair rings: slot = t%2
            a2r = [rp.tile([104, WIN], dt.bfloat16, name=f"a2r{i}")
                   for i in range(2)]
            of3 = [rp.tile([36, NT], dt.bfloat16, name=f"of3{i}")
                   for i in range(2)]
            nbt = [rp.tile([36, NT], dt.float32, name=f"nbt{i}")
                   for i in range(2)]

            SPLIT_COL = 4039   # piece A depends only on the o2d T half

            def prefetch_S(s, dup_dram=True):
                """window DMAs (from DRAM staging) building S for sample s,
                rows c*26+w plus the row-64 duplicate, split into column
                pieces across the sync/gpsimd queues (per-queue DMAs
                serialize at ~34GB/s, so pieces parallelize)."""
                St = S_bufs[s % 4]
                qs = [nc.sync, nc.gpsimd]
                qi = s
                for c in range(2):
                    for dd in range(2):
                        r0 = 64 * dd + c * 26
                        for (u0, u1) in ((0, SPLIT_COL), (SPLIT_COL, LC)):
                            win = o2d[s * 2 + c:s * 2 + c + 1, u0:u1].copy()
                            win.ap = bass_rust.VecI64Pair(
                                [[L, 1], [1, 26], [1, u1 - u0]])
                            qs[qi % 2].dma_start(
                                out=St[r0:r0 + 26, u0:u1], in_=win)
                            qi += 1

            def halves(ln):
                out = [(0, min(NT, ln))]
                if ln > NT:
                    out.append((NT, ln))
                return out

            # ---------------- stage emitters ----------------
            def dil_mm(s, j, half):
                ln = _wlen('dil', j)
                hs = halves(ln)
                if half >= len(hs):
                    return
                lo, hi = hs[half]
                St = S_bufs[s % 4]
                t0 = STEP * j
                nc.tensor.matmul(P_DA[0:100, lo:hi], W['dilA'][...],
                                 St[0:52, t0 + lo:t0 + hi],
                                 start=True, stop=True, tile_position=(0, 0),
                                 skip_group_check=True)
                nc.tensor.matmul(P_DB[0:100, 0:hi - lo], W['dilB'][64:116],
                                 St[64:116, t0 + lo:t0 + hi],
                                 start=True, stop=True, tile_position=(64, 0),
                                 skip_group_check=True)

            def dil_post(s, j, slot, half):
                ln = _wlen('dil', j)
                hs = halves(ln)
                if half >= len(hs):
                    return
                lo, hi = hs[half]
                nc.vector.tensor_scalar(Ob[slot][:, lo:hi],
                                        P_DB[0:100, 0:hi - lo],
                                        W['bMB'][...], 0.0, ALU.add, ALU.max)
                if half == len(hs) - 1:
                    nc.scalar.activation(Oa[slot][:, 0:ln], P_DA[0:100, 0:ln],
                                         AF.Relu, bias=W['bMA'][...])

            def f1_mm(s, j, slot, half):
                ln = _wlen('f1', j)
                hs = halves(ln)
                if half >= len(hs):
                    return
                lo, hi = hs[half]
                for dw in range(6):
                    nc.tensor.matmul(P_F1[0:48, lo:hi], W['F1'][dw][...],
                                     Oa[slot][:, lo + dw:hi + dw],
                                     start=(dw == 0), stop=(dw == 5),
                                     skip_group_check=True)
                    nc.tensor.matmul(P_F1[64:112, lo:hi], W['F1'][dw][...],
                                     Ob[slot][:, lo + dw:hi + dw],
                                     start=(dw == 0), stop=(dw == 5),
                                     skip_group_check=True)

            def f1_post(s, j, slot):
                ln = _wlen('f1', j)
                nc.scalar.activation(a1r[slot][:, 0:ln], P_F1[0:112, 0:ln],
                                     AF.Sigmoid, bias=W['bF1'][...])

            def f2_mm(pair, j, t, half):
                ln = _wlen('f2', j)
                hs = halves(ln)
                if half >= len(hs):
                    return
                lo, hi = hs[half]
                sl0 = 2 * 0 + (t % 2)   # sample 2*pair
                sl1 = 2 * 1 + (t % 2)   # sample 2*pair+1
                for dw in range(6):
                    nc.tensor.matmul(P_F2[0:40, lo:hi], W['F2'][dw][...],
                                     a1r[sl0][:, lo + dw:hi + dw],
                                     start=(dw == 0), stop=(dw == 5),
                                     skip_group_check=True)
                    nc.tensor.matmul(P_F2[64:104, lo:hi], W['F2'][dw][...],
                                     a1r[sl1][:, lo + dw:hi + dw],
                                     start=(dw == 0), stop=(dw == 5),
                                     skip_group_check=True)

            def f2_post(pair, j, t):
                ln = _wlen('f2', j)
                slot = t % 2
                nc.scalar.activation(a2r[slot][:, 0:ln], P_F2[0:104, 0:ln],
                                     AF.Sigmoid, bias=W['bF2'][...])

            def f3_half(pair, j, t, half):
                ln = _wlen('f3', j)
                hs = halves(ln)
                if half >= len(hs):
                    return
                lo, hi = hs[half]
                slot = t % 2
                t0 = STEP * j
                # 4-way packed: pA chains (dw 0..2) at partitions 0..3 (s0)
                # and 32..35 (s1); nB chains (negated dw 3..5) at 64..67 /
                # 96..99; fused (pA - thr) > nB threshold
                for dwl in range(3):
                    for si in range(2):
                        dwa, dwb = dwl, 3 + dwl
                        rhs = a2r[slot][64 * si:64 * si + 40, :]
                        wA = W['F3'][dwa][...] if si == 0 \
                            else W['F3d'][dwa][64:104]
                        wB = W['F3'][dwb][...] if si == 0 \
                            else W['F3d'][dwb][64:104]
                        nc.tensor.matmul(
                            P_F3[32 * si:32 * si + 4, 0:hi - lo], wA,
                            rhs[:, lo + dwa:hi + dwa],
                            start=(dwl == 0), stop=(dwl == 2),
                            tile_position=(64 * si, 32 * si),
                            skip_group_check=True)
                        nc.tensor.matmul(
                            P_F3[64 + 32 * si:64 + 32 * si + 4, 0:hi - lo], wB,
                            rhs[:, lo + dwb:hi + dwb],
                            start=(dwl == 0), stop=(dwl == 2),
                            tile_position=(64 * si, 64 + 32 * si),
                            skip_group_check=True)
                nc.scalar.activation(nbt[slot][:, 0:hi - lo],
                                     P_F3[64:100, 0:hi - lo], AF.Copy)
                nc.vector.scalar_tensor_tensor(
                    of3[slot][:, 0:hi - lo], P_F3[0:36, 0:hi - lo],
                    W['thr'][...], nbt[slot][:, 0:hi - lo],
                    ALU.subtract, ALU.is_gt)
                for si in range(2):
                    s = pair * 2 + si
                    nc.gpsimd.dma_start(
                        out=out_d[s * 4:(s + 1) * 4, t0 + lo:t0 + hi],
                        in_=of3[slot][si * 32:si * 32 + 4, 0:hi - lo])

            # ---------------- skewed global pipeline ----------------
            NTOT = NPAIR * NSUP           # 36 global supertiles
            for s in (0, 1, 2, 3):
                prefetch_S(s, dup_dram=(s < 2))

            def stage_pt(g, skew):
                tt = g - skew
                if 0 <= tt < NTOT:
                    return tt, tt // NSUP, tt % NSUP
                return None, None, None

            for g in range(NTOT + 3):
                tD, pD, jD = stage_pt(g, 0)
                t1_, p1, j1 = stage_pt(g, 1)
                t2_, p2, j2 = stage_pt(g, 2)
                t3_, p3, j3 = stage_pt(g, 3)

                # S prefetch for the next pair, early in the pair
                if tD is not None and jD == 2 and pD < NPAIR - 1:
                    prefetch_S(2 * (pD + 1))
                    prefetch_S(2 * (pD + 1) + 1)

                sD0 = 2 * pD if tD is not None else None
                sD1 = sD0 + 1 if tD is not None else None
                slD0 = (tD % 2) if tD is not None else None
                slD1 = 2 + (tD % 2) if tD is not None else None
                sF0 = 2 * p1 if t1_ is not None else None
                sF1 = sF0 + 1 if t1_ is not None else None
                slF0 = (t1_ % 2) if t1_ is not None else None
                slF1 = 2 + (t1_ % 2) if t1_ is not None else None

                if tD is not None:
                    dil_mm(sD0, jD, 0)
                if t1_ is not None:
                    f1_mm(sF0, j1, slF0, 0)
                if tD is not None:
                    dil_post(sD0, jD, slD0, 0)
                    dil_mm(sD0, jD, 1)
                if t2_ is not None:
                    f2_mm(p2, j2, t2_, 0)
                if t1_ is not None:
                    f1_mm(sF0, j1, slF0, 1)
                    f1_post(sF0, j1, slF0)
                if tD is not None:
                    dil_post(sD0, jD, slD0, 1)
                if t3_ is not None:
                    f3_half(p3, j3, t3_, 0)
                if tD is not None:
                    dil_mm(sD1, jD, 0)
                if t2_ is not None:
                    f2_mm(p2, j2, t2_, 1)
                if t1_ is not None:
                    f1_mm(sF1, j1, slF1, 0)
                if tD is not None:
                    dil_post(sD1, jD, slD1, 0)
                    dil_mm(sD1, jD, 1)
                if t1_ is not None:
                    f1_mm(sF1, j1, slF1, 1)
                    f1_post(sF1, j1, slF1)
                if tD is not None:
                    dil_post(sD1, jD, slD1, 1)
                if t2_ is not None:
                    f2_post(p2, j2, t2_)
                if t3_ is not None:
                    f3_half(p3, j3, t3_, 1)

    return _patch_serialization(nc)


def kernel(**inputs):
    inputs = {k: np.asarray(v) for k, v in inputs.items()}
    in_maps = _shard_inputs(inputs)
    nc = build_bass()
    from concourse.bass_utils import run_bass_kernel_spmd
    res = run_bass_kernel_spmd(nc, in_maps, core_ids=list(range(NCORES)))
    outs = [res.results[i]['out'].reshape(BS, 4, T3) for i in range(NCORES)]
    full = np.concatenate(outs, axis=0)[:, None]  # [64, 1, 4, T3]
    return full.astype(np.float32)
